# revision 2
# baseline (speedup 1.0000x reference)
"""Trainium2 Bass kernel for the Nawrot downsampler-upsampler module.

Per-core (data-parallel over batch, 1 example per NeuronCore):
  1. PE per-tile prefix sums of x along L (triangular matmul, NO cross-tile
     carry: the carry is folded into the final selection matmul as a +1
     coefficient on the last row of the previous tile) -> P_loc kept in SBUF
  2. PE-transpose x -> fp32 MLP (relu(x@W1+b1)) -> logits via ones-matmul
     partition reduce
  3. boundary bits from logits + logistic noise; segment scans via
     tensor_tensor_scan + cross-partition max combine
  4. final: per 128-token tile, build a +-1 one-hot coefficient matrix
     (segment end minus segment start-1, window = prev tile + cur tile +
     null row) and contract it against the SBUF-resident P_loc tiles with
     matmuls; scale by 1/count on the scalar engine.

Matmuls run as float32r (bit-identical layout to fp32; 1 cycle/row when the
moving dim >= 256 vs fp32's 4).
"""
import sys

sys.path.insert(0, "/opt/trn_rl_repo")

import numpy as np
from contextlib import ExitStack

import concourse.bass as bass
import concourse.bacc as bacc
import concourse.tile as tile
from concourse import mybir
from concourse.masks import make_identity

F32 = mybir.dt.float32
F32R = mybir.dt.float32r
I32 = mybir.dt.int32
OP = mybir.AluOpType
ACT = mybir.ActivationFunctionType

B = 8
L_FULL = 2048
D_FULL = 1024
N_CORES = 8

# fp32r toggles for bisection (all matmul groups have >=256 moving rows)
F32R_MLP = True   # x transposes + x@W1 + logits partition-reduce
F32R_PRE = True   # per-tile triangular prefix matmuls
F32R_FIN = True   # final one-hot selection matmuls + CT transposes


def _r(ap, on):
    return ap.bitcast(F32R) if on else ap


def build(L=L_FULL, D=D_FULL):
    P = 128
    NLT = L // P          # 128-row l-tiles
    ND = D // P           # d-tiles
    CPT = L // P          # scan columns per partition (l = p*CPT + c)
    DC = min(512, D)      # matmul free-dim chunk
    NDC = D // DC
    LCH = min(512, L)     # l-chunk for MLP matmuls
    NLC = L // LCH
    WIN = 257             # selection window: prev tile (128) + cur (128) + null

    nc = bacc.Bacc("TRN2", target_bir_lowering=False, debug=False, num_devices=N_CORES)

    x_d = nc.dram_tensor("x", [L, D], F32, kind="ExternalInput").ap()
    noise_d = nc.dram_tensor("noise", [L], F32, kind="ExternalInput").ap()
    w1_d = nc.dram_tensor("W1", [D, D], F32, kind="ExternalInput").ap()
    b1_d = nc.dram_tensor("b1", [D], F32, kind="ExternalInput").ap()
    w2_d = nc.dram_tensor("W2", [D], F32, kind="ExternalInput").ap()
    b2_d = nc.dram_tensor("b2", [1], F32, kind="ExternalInput").ap()
    null_d = nc.dram_tensor("null_group", [1, 1, D], F32, kind="ExternalInput").ap()
    up_d = nc.dram_tensor("up", [L, D], F32, kind="ExternalOutput").ap()

    with tile.TileContext(nc) as tc, ExitStack() as ctx:
        const = ctx.enter_context(tc.tile_pool(name="const", bufs=1))
        dram = ctx.enter_context(tc.tile_pool(name="dram", bufs=1, space="DRAM"))
        xpool = ctx.enter_context(tc.tile_pool(name="xp", bufs=3))
        xtp = ctx.enter_context(tc.tile_pool(name="xtp", bufs=2))
        stage = ctx.enter_context(tc.tile_pool(name="stage", bufs=3))
        small = ctx.enter_context(tc.tile_pool(name="small", bufs=1))
        gpool = ctx.enter_context(tc.tile_pool(name="gp", bufs=2))
        logp = ctx.enter_context(tc.tile_pool(name="logp", bufs=2))
        psA = ctx.enter_context(tc.tile_pool(name="psA", bufs=2, space="PSUM"))
        psT = ctx.enter_context(tc.tile_pool(name="psT", bufs=2, space="PSUM"))
        psM = ctx.enter_context(tc.tile_pool(name="psM", bufs=2, space="PSUM"))

        # ---------------- DRAM scratch ----------------
        lg_scr = dram.tile([L, 1], F32)      # logits row bounce
        pe_scr = dram.tile([L, 1], F32)      # per-token segment-end row index
        ps_scr = dram.tile([L, 1], F32)      # per-token segment-start-minus-one row index
        r_scr = dram.tile([L, 1], F32)       # per-token reciprocal count

        # ---------------- constants ----------------
        const_dmas = []
        w1_sb = const.tile([P, ND, D], F32)
        const_dmas.append(nc.sync.dma_start(out=w1_sb[:], in_=w1_d.rearrange("(i p) n -> p i n", p=P)))
        b1_sb = const.tile([P, ND], F32)
        const_dmas.append(nc.sync.dma_start(out=b1_sb[:], in_=b1_d.rearrange("(o p) -> p o", p=P)))
        w2_sb = const.tile([P, ND], F32)
        const_dmas.append(nc.sync.dma_start(out=w2_sb[:], in_=w2_d.rearrange("(o p) -> p o", p=P)))
        b2_sb = const.tile([1, 1], F32)
        const_dmas.append(nc.sync.dma_start(out=b2_sb[:], in_=b2_d.rearrange("(a b) -> a b", a=1)))
        null_sb = const.tile([1, D], F32)
        const_dmas.append(nc.sync.dma_start(out=null_sb[:], in_=null_d[0, 0, :].rearrange("(a d) -> a d", a=1)))

        # P_loc prefix tiles, SBUF-resident across the whole kernel
        pbig = const.tile([P, NLT, D], F32)

        ident = const.tile([P, P], F32)
        make_identity(nc, ident[:])

        pio = const.tile([P, 1], F32)
        nc.gpsimd.iota(pio[:], pattern=[[0, 1]], base=0, channel_multiplier=1,
                       allow_small_or_imprecise_dtypes=True)
        fio = const.tile([P, P], F32)
        nc.gpsimd.iota(fio[:], pattern=[[1, P]], base=0, channel_multiplier=0,
                       allow_small_or_imprecise_dtypes=True)
        # ut[k, m] = 1 if k <= m   (inclusive prefix lhsT)
        ut = const.tile([P, P], F32)
        nc.vector.tensor_scalar(out=ut[:], in0=fio[:], scalar1=pio[:], scalar2=None, op0=OP.is_ge)
        ones_col = const.tile([P, 1], F32)
        nc.vector.memset(ones_col[:], 1.0)
        ones_1x1 = const.tile([1, 1], F32)
        nc.vector.memset(ones_1x1[:], 1.0)
        zeros_cpt = const.tile([P, CPT], F32)
        nc.vector.memset(zeros_cpt[:], 0.0)
        zrow128 = const.tile([1, P], F32)
        nc.vector.memset(zrow128[:], 0.0)
        iotp1 = const.tile([P, CPT], F32)   # l + 1 (l = p*CPT + c), exact in f32
        nc.gpsimd.iota(iotp1[:], pattern=[[1, CPT]], base=1, channel_multiplier=CPT,
                       allow_small_or_imprecise_dtypes=True)
        # window iota: cols 0..255 hold 0..255 (local row index), col 256 is the
        # null slot (matched by clamped pe for null tokens)
        iwin = const.tile([P, WIN], F32)
        nc.gpsimd.iota(iwin[:], pattern=[[1, WIN]], base=0, channel_multiplier=0,
                       allow_small_or_imprecise_dtypes=True)

        # Collapse the fan of constant-load DMA lanes into one tick so later
        # matmuls don't exceed the per-instruction sync-wait slot limit.  The
        # barrier NOP itself is subject to the same limit, so first absorb the
        # DMA-lane semaphores into the SP clock with nops of <=4 deps each.
        from concourse.tile_rust import add_dep_helper as _adh
        for g in range(0, len(const_dmas), 4):
            spn = nc.sync.nop()
            for d in const_dmas[g:g + 4]:
                _adh(spn.ins, d.ins, sync=True, reason="const-lane coalesce")
        tc.strict_bb_all_engine_barrier()

        # ------- phases 1+2: per 512-token chunk: load, transpose, prefix, MLP -------
        for lc in range(NLC):
            lsl = slice(lc * LCH, (lc + 1) * LCH)
            xT_ch = xtp.tile([P, ND, LCH], F32, tag="xT")  # xT[p, j, l_local]

            for ii in range(LCH // P):
                i = lc * (LCH // P) + ii
                x_t = xpool.tile([P, D], F32, tag="x")
                nc.sync.dma_start(out=x_t[:], in_=x_d[i * P:(i + 1) * P, :])

                # transposes in groups of 4 per PSUM bank
                for jg in range((ND + 3) // 4):
                    n_in_g = min(4, ND - jg * 4)
                    ps_t = psT.tile([P, 512], F32, tag="tr")
                    for jj in range(n_in_g):
                        j = jg * 4 + jj
                        nc.tensor.transpose(
                            out=ps_t[:, jj * P:(jj + 1) * P],
                            in_=_r(x_t[:, j * P:(j + 1) * P], F32R_MLP),
                            identity=_r(ident[:], F32R_MLP),
                        )
                    nc.vector.tensor_copy(
                        out=xT_ch[:, jg * 4:jg * 4 + n_in_g, ii * P:(ii + 1) * P],
                        in_=ps_t[:, :n_in_g * P].rearrange("p (j q) -> p j q", q=P),
                    )

                # per-tile prefix (no cross-tile carry; folded into phase 4)
                psP = psA.tile([P, D], F32, tag="P")
                for dc in range(NDC):
                    sl = slice(dc * DC, (dc + 1) * DC)
                    nc.tensor.matmul(
                        psP[:, sl], lhsT=_r(ut[:], F32R_PRE), rhs=_r(x_t[:, sl], F32R_PRE),
                        start=True, stop=True,
                    )
                nc.vector.tensor_copy(out=pbig[:, i, :], in_=psP[:])

            # MLP for this l-chunk
            logacc = logp.tile([P, LCH], F32, tag="logacc")
            for o in range(ND):
                psm = psM.tile([P, LCH], F32, tag="mlp")
                for i_ in range(ND):
                    nc.tensor.matmul(
                        psm[:],
                        lhsT=_r(w1_sb[:, i_, o * P:(o + 1) * P], F32R_MLP),
                        rhs=_r(xT_ch[:, i_, :], F32R_MLP),
                        start=(i_ == 0), stop=(i_ == ND - 1),
                    )
                hT = stage.tile([P, LCH], F32, tag="hT")
                nc.scalar.activation(
                    out=hT[:], in_=psm[:], func=ACT.Relu,
                    bias=b1_sb[:, o:o + 1], scale=1.0,
                )
                if o == 0:
                    nc.vector.tensor_scalar(
                        out=logacc[:], in0=hT[:],
                        scalar1=w2_sb[:, o:o + 1], scalar2=None, op0=OP.mult,
                    )
                else:
                    nc.vector.scalar_tensor_tensor(
                        out=logacc[:], in0=hT[:], scalar=w2_sb[:, o:o + 1],
                        in1=logacc[:], op0=OP.mult, op1=OP.add,
                    )

            # logits partial for this chunk: partition-reduce + bias, to DRAM
            pslg = psM.tile([1, LCH], F32, tag="mlp")
            nc.tensor.matmul(pslg[:], lhsT=_r(ones_col[:], F32R_MLP),
                             rhs=_r(logacc[:], F32R_MLP), start=True, stop=True)
            lg_ch = stage.tile([1, LCH], F32, tag="lgch")
            nc.scalar.activation(
                out=lg_ch[:], in_=pslg[:], func=ACT.Identity,
                bias=b2_sb[:, 0:1], scale=1.0,
            )
            nc.sync.dma_start(
                out=lg_scr[lsl, 0].rearrange("(a l) -> a l", a=1), in_=lg_ch[:]
            )

        # ---------------- phase 3: boundary bits, cumsum ----------------
        lg16 = small.tile([P, CPT], F32, tag="lg16")
        nc.sync.dma_start(out=lg16[:], in_=lg_scr[:, 0].rearrange("(p c) -> p c", c=CPT))

        nz16 = small.tile([P, CPT], F32, tag="nz")
        nc.sync.dma_start(out=nz16[:], in_=noise_d.rearrange("(p c) -> p c", c=CPT))

        lnu = small.tile([P, CPT], F32, tag="lnu")
        nc.scalar.activation(out=lnu[:], in_=nz16[:], func=ACT.Ln)
        om = small.tile([P, CPT], F32, tag="om")
        nc.vector.tensor_scalar(
            out=om[:], in0=nz16[:], scalar1=1.0, scalar2=-1.0,
            op0=OP.subtract, op1=OP.mult,
        )  # (u - 1) * -1 = 1 - u
        ln1m = small.tile([P, CPT], F32, tag="ln1m")
        nc.scalar.activation(out=ln1m[:], in_=om[:], func=ACT.Ln)
        tt = small.tile([P, CPT], F32, tag="tt")
        nc.vector.tensor_tensor(out=tt[:], in0=lnu[:], in1=ln1m[:], op=OP.subtract)
        nc.vector.tensor_tensor(out=tt[:], in0=tt[:], in1=lg16[:], op=OP.add)
        hard = small.tile([P, CPT], F32, tag="hard")
        nc.vector.tensor_scalar(out=hard[:], in0=tt[:], scalar1=0.0, scalar2=None, op0=OP.is_gt)

        # ---- prefix-max scans: lb_inc (last boundary <= l), scan2 (boundary before it)
        def cross_part_max_scan(inclusive, tagp):
            """Combine per-partition inclusive max-scans into a global scan.

            Returns a (P, CPT) tile where each row has been max-ed with the
            running max of all previous partitions' row-maxima.
            """
            # row maxima -> (1, P) via matmul with identity rhs
            ps_r = psT.tile([P, 512], F32, tag="tr")
            nc.tensor.matmul(
                ps_r[0:1, 0:P], lhsT=inclusive[:, CPT - 1:CPT], rhs=ident[:],
                start=True, stop=True,
            )
            rowT = small.tile([1, P], F32, tag=tagp + "_rowT")
            nc.vector.tensor_copy(out=rowT[:], in_=ps_r[0:1, 0:P])
            # inclusive scan along the (1, P) row, then shift right one (exclusive)
            sc = small.tile([1, P], F32, tag=tagp + "_sc")
            nc.vector.tensor_tensor_scan(
                out=sc[:], data0=rowT[:], data1=zrow128[:],
                initial=-1.0, op0=OP.max, op1=OP.add,
            )
            exc = small.tile([1, P], F32, tag=tagp + "_exc")
            nc.vector.memset(exc[0:1, 0:1], -1.0)
            nc.vector.tensor_copy(out=exc[0:1, 1:P], in_=sc[0:1, 0:P - 1])
            # back to (P, 1) via rank-1 matmul with ones (1,1)
            ps_b = psT.tile([P, 512], F32, tag="tr")
            nc.tensor.matmul(
                ps_b[:, 0:1], lhsT=exc[:], rhs=ones_1x1[:], start=True, stop=True,
            )
            offm = small.tile([P, 1], F32, tag=tagp + "_offm")
            nc.vector.tensor_copy(out=offm[:], in_=ps_b[:, 0:1])
            out_t = small.tile([P, CPT], F32, tag=tagp + "_out")
            nc.vector.tensor_scalar(
                out=out_t[:], in0=inclusive[:], scalar1=offm[:], scalar2=None, op0=OP.max,
            )
            return out_t, offm

        # mi = hard ? l : -1  == (l+1)*hard - 1
        mi = small.tile([P, CPT], F32, tag="mi")
        nc.vector.tensor_tensor(out=mi[:], in0=iotp1[:], in1=hard[:], op=OP.mult)
        nc.vector.tensor_scalar(out=mi[:], in0=mi[:], scalar1=-1.0, scalar2=None, op0=OP.add)
        s1l = small.tile([P, CPT], F32, tag="s1l")
        nc.vector.tensor_tensor_scan(
            out=s1l[:], data0=mi[:], data1=zeros_cpt[:],
            initial=-1.0, op0=OP.max, op1=OP.add,
        )
        lb_inc, offm1 = cross_part_max_scan(s1l, "s1")

        # lbm1[l] = lb_inc[l-1] (token shift; layout l = p*CPT + c).
        # Column 0 of partition p is lb_inc at the end of partition p-1,
        # which is exactly the exclusive cross-partition max offm1.
        lbm1 = small.tile([P, CPT], F32, tag="lbm1")
        nc.vector.tensor_copy(out=lbm1[:, 0:1], in_=offm1[:])
        nc.vector.tensor_copy(out=lbm1[:, 1:CPT], in_=lb_inc[:, 0:CPT - 1])
        # mi2 = hard ? lbm1 : -1 == (lbm1+1)*hard - 1
        mi2 = small.tile([P, CPT], F32, tag="mi2")
        nc.vector.tensor_scalar(out=mi2[:], in0=lbm1[:], scalar1=1.0, scalar2=None, op0=OP.add)
        nc.vector.tensor_tensor(out=mi2[:], in0=mi2[:], in1=hard[:], op=OP.mult)
        nc.vector.tensor_scalar(out=mi2[:], in0=mi2[:], scalar1=-1.0, scalar2=None, op0=OP.add)
        s2l = small.tile([P, CPT], F32, tag="s2l")
        nc.vector.tensor_tensor_scan(
            out=s2l[:], data0=mi2[:], data1=zeros_cpt[:],
            initial=-1.0, op0=OP.max, op1=OP.add,
        )
        pb, _ = cross_part_max_scan(s2l, "s2")

        # cnt = lb_inc - pb ;  r = 1/(cnt + 1e-9), forced to 1.0 for null tokens
        cnt = small.tile([P, CPT], F32, tag="cnt")
        nc.vector.tensor_tensor(out=cnt[:], in0=lb_inc[:], in1=pb[:], op=OP.subtract)
        nc.vector.tensor_scalar(out=cnt[:], in0=cnt[:], scalar1=1e-9, scalar2=None, op0=OP.add)
        r_tok = small.tile([P, CPT], F32, tag="r_tok")
        nc.vector.reciprocal(out=r_tok[:], in_=cnt[:])
        mask0 = small.tile([P, CPT], F32, tag="mask0")
        nc.vector.tensor_scalar(out=mask0[:], in0=lb_inc[:], scalar1=-0.5, scalar2=None, op0=OP.is_gt)
        # r_tok = (r_tok - 1)*mask0 + 1
        nc.vector.tensor_scalar(out=r_tok[:], in0=r_tok[:], scalar1=-1.0, scalar2=None, op0=OP.add)
        nc.vector.tensor_tensor(out=r_tok[:], in0=r_tok[:], in1=mask0[:], op=OP.mult)
        nc.vector.tensor_scalar(out=r_tok[:], in0=r_tok[:], scalar1=1.0, scalar2=None, op0=OP.add)
        # pe = mask0 ? lb_inc : L+1 (null slot)   == (lb_inc - (L+1))*mask0 + (L+1)
        pe_t = small.tile([P, CPT], F32, tag="pe_t")
        nc.vector.tensor_scalar(out=pe_t[:], in0=lb_inc[:], scalar1=-float(L + 1), scalar2=None, op0=OP.add)
        nc.vector.tensor_tensor(out=pe_t[:], in0=pe_t[:], in1=mask0[:], op=OP.mult)
        nc.vector.tensor_scalar(out=pe_t[:], in0=pe_t[:], scalar1=float(L + 1), scalar2=None, op0=OP.add)
        # ps = pb >= 0 ? pb : L (zero contribution)  == (pb - L)*mask2 + L
        mask2 = small.tile([P, CPT], F32, tag="mask2")
        nc.vector.tensor_scalar(out=mask2[:], in0=pb[:], scalar1=-0.5, scalar2=None, op0=OP.is_gt)
        ps_t2 = small.tile([P, CPT], F32, tag="ps_t2")
        nc.vector.tensor_scalar(out=ps_t2[:], in0=pb[:], scalar1=-float(L), scalar2=None, op0=OP.add)
        nc.vector.tensor_tensor(out=ps_t2[:], in0=ps_t2[:], in1=mask2[:], op=OP.mult)
        nc.vector.tensor_scalar(out=ps_t2[:], in0=ps_t2[:], scalar1=float(L), scalar2=None, op0=OP.add)

        # layout bounce (p*CPT+c) -> (128t+p) chunked, all f32
        nc.sync.dma_start(out=pe_scr[:, 0].rearrange("(p c) -> p c", c=CPT), in_=pe_t[:])
        nc.sync.dma_start(out=ps_scr[:, 0].rearrange("(p c) -> p c", c=CPT), in_=ps_t2[:])
        nc.sync.dma_start(out=r_scr[:, 0].rearrange("(p c) -> p c", c=CPT), in_=r_tok[:])
        pe2 = small.tile([P, NLT], F32, tag="pe2")
        nc.sync.dma_start(out=pe2[:], in_=pe_scr[:, 0].rearrange("(t p) -> p t", p=P))
        ps2 = small.tile([P, NLT], F32, tag="ps2")
        nc.sync.dma_start(out=ps2[:], in_=ps_scr[:, 0].rearrange("(t p) -> p t", p=P))
        r2 = small.tile([P, NLT], F32, tag="r2")
        nc.sync.dma_start(out=r2[:], in_=r_scr[:, 0].rearrange("(t p) -> p t", p=P))

        # ---- final: per 128-token tile, one-hot selection matmul over the
        # window [prev tile rows | cur tile rows | null slot] of P_loc ----
        for t in range(NLT):
            base = (t - 1) * P  # global row index of window col 0

            # local window indices; clamp pe to the null slot (256).
            # pe in {0..L-1} U {L+1}; normal pe-base <= 255, null >= 257 -> 256.
            pel = gpool.tile([P, 1], F32, tag="pel")
            nc.vector.tensor_scalar(
                out=pel[:], in0=pe2[:, t:t + 1], scalar1=-float(base),
                scalar2=float(WIN - 1), op0=OP.add, op1=OP.min,
            )
            # ps in {0..L-1} U {L}; ps=L (zero row) must match nothing: no clamp
            psl = gpool.tile([P, 1], F32, tag="psl")
            nc.vector.tensor_scalar(
                out=psl[:], in0=ps2[:, t:t + 1], scalar1=-float(base),
                scalar2=None, op0=OP.add,
            )

            ct = gpool.tile([P, WIN], F32, tag="ct")
            ct2 = gpool.tile([P, WIN], F32, tag="ct2")
            nc.vector.tensor_scalar(out=ct[:], in0=iwin[:], scalar1=pel[:], scalar2=None, op0=OP.is_equal)
            nc.vector.tensor_scalar(out=ct2[:], in0=iwin[:], scalar1=psl[:], scalar2=None, op0=OP.is_equal)
            nc.vector.tensor_tensor(out=ct[:], in0=ct[:], in1=ct2[:], op=OP.subtract)

            if t > 0:
                # cross-tile carry: pe in cur tile & ps in prev tile ->
                # +1 on the last row of the prev tile (adds its row-sum,
                # i.e. the inter-tile offset difference)
                crA = gpool.tile([P, 1], F32, tag="crA")
                nc.vector.tensor_scalar(out=crA[:], in0=pe2[:, t:t + 1],
                                        scalar1=float(base + P), scalar2=None, op0=OP.is_ge)
                crB = gpool.tile([P, 1], F32, tag="crB")
                nc.vector.tensor_scalar(out=crB[:], in0=ps2[:, t:t + 1],
                                        scalar1=float(base + P), scalar2=None, op0=OP.is_lt)
                nc.vector.tensor_tensor(out=crA[:], in0=crA[:], in1=crB[:], op=OP.mult)
                nc.vector.tensor_tensor(out=ct[:, P - 1:P], in0=ct[:, P - 1:P], in1=crA[:], op=OP.add)

            # transpose CT -> C chunks (lhsT for the selection matmuls)
            ps_c = psT.tile([P, 512], F32, tag="tr")
            if t > 0:
                nc.tensor.transpose(out=ps_c[:, 0:P], in_=_r(ct[:, 0:P], F32R_FIN),
                                    identity=_r(ident[:], F32R_FIN))
            nc.tensor.transpose(out=ps_c[:, P:2 * P], in_=_r(ct[:, P:2 * P], F32R_FIN),
                                identity=_r(ident[:], F32R_FIN))
            if t == 0:
                nc.tensor.transpose(out=ps_c[0:1, 2 * P:3 * P], in_=_r(ct[:, 2 * P:2 * P + 1], F32R_FIN),
                                    identity=_r(ident[:], F32R_FIN))
            c_sb = gpool.tile([P, 3 * P], F32, tag="c_sb")
            if t > 0:
                nc.vector.tensor_copy(out=c_sb[:, 0:2 * P], in_=ps_c[:, 0:2 * P])
            else:
                nc.vector.tensor_copy(out=c_sb[:, P:2 * P], in_=ps_c[:, P:2 * P])
                nc.vector.tensor_copy(out=c_sb[0:1, 2 * P:3 * P], in_=ps_c[0:1, 2 * P:3 * P])

            psO = psA.tile([P, D], F32, tag="P")
            for dc in range(NDC):
                sl = slice(dc * DC, (dc + 1) * DC)
                if t > 0:
                    nc.tensor.matmul(psO[:, sl], lhsT=_r(c_sb[:, 0:P], F32R_FIN),
                                     rhs=_r(pbig[:, t - 1, sl], F32R_FIN),
                                     start=True, stop=False)
                    nc.tensor.matmul(psO[:, sl], lhsT=_r(c_sb[:, P:2 * P], F32R_FIN),
                                     rhs=_r(pbig[:, t, sl], F32R_FIN),
                                     start=False, stop=True)
                else:
                    nc.tensor.matmul(psO[:, sl], lhsT=_r(c_sb[:, P:2 * P], F32R_FIN),
                                     rhs=_r(pbig[:, t, sl], F32R_FIN),
                                     start=True, stop=False)
                    nc.tensor.matmul(psO[:, sl], lhsT=_r(c_sb[0:1, 2 * P:3 * P], F32R_FIN),
                                     rhs=_r(null_sb[0:1, sl], F32R_FIN),
                                     start=False, stop=True)

            upt = stage.tile([P, D], F32, tag="up")
            nc.scalar.activation(out=upt[:], in_=psO[:], func=ACT.Copy,
                                 bias=0.0, scale=r2[:, t:t + 1])
            nc.sync.dma_start(out=up_d[t * P:(t + 1) * P, :], in_=upt[:])

    nc.compile()
    return nc


_nc_cache = {}


def _get_nc(L, D):
    key = (L, D)
    if key not in _nc_cache:
        _nc_cache[key] = build(L, D)
    return _nc_cache[key]


def make_in_maps(inputs, n_cores=N_CORES):
    x = np.ascontiguousarray(np.asarray(inputs["x"], dtype=np.float32))
    noise = np.ascontiguousarray(np.asarray(inputs["noise"], dtype=np.float32))
    shared = {
        "W1": np.ascontiguousarray(np.asarray(inputs["W1"], dtype=np.float32)),
        "b1": np.ascontiguousarray(np.asarray(inputs["b1"], dtype=np.float32)),
        "W2": np.ascontiguousarray(np.asarray(inputs["W2"], dtype=np.float32)),
        "b2": np.ascontiguousarray(np.asarray(inputs["b2"], dtype=np.float32)),
        "null_group": np.ascontiguousarray(np.asarray(inputs["null_group"], dtype=np.float32)),
    }
    return [dict(shared, x=x[c], noise=noise[c]) for c in range(n_cores)]


def kernel(**inputs):
    from concourse.bass_utils import run_bass_kernel_spmd

    x = np.asarray(inputs["x"])
    b, L, D = x.shape
    assert b == N_CORES
    nc = _get_nc(L, D)
    in_maps = make_in_maps(inputs)
    res = run_bass_kernel_spmd(nc, in_maps, core_ids=list(range(N_CORES)))
    out = np.stack([res.results[c]["up"] for c in range(N_CORES)], axis=0)
    return out.astype(np.float32)


# revision 7
# speedup vs baseline: 1.6619x; 1.6619x over previous
"""Trainium2 Bass kernel for the Nawrot downsampler-upsampler module.

Per-core (data-parallel over batch, 1 example per NeuronCore):
  1. PE per-tile prefix sums of x along L (triangular fp32r matmul, NO
     cross-tile carry: the carry is folded into the final selection matmul
     as a +1 coefficient on the last row of the previous tile) -> P_loc
     kept in SBUF.
  2. MLP relu(x@W1+b1) via 3-pass split-bf16 matmuls (x = xh+xl, W1 =
     wh+wl; xh*wh + xh*wl + xl*wh reproduces fp32 to ~2^-16) over
     host-pretransposed xT; logits via ones-matmul partition reduce.
  3. boundary bits from logits + logistic noise; segment scans via
     tensor_tensor_scan + cross-partition max combine.
  4. final: per 128-token tile, build a +-1 one-hot coefficient matrix
     (segment end minus segment start-1, window = prev tile + cur tile +
     null slot) and contract it against the SBUF-resident P_loc tiles with
     fp32r matmuls; scale by 1/count on the scalar engine.

fp32r (4-byte, ~12 mantissa bits, 1 cycle/row vs fp32's 4) is used where
the ~2e-4 rounding is harmless (segment averages); the logits path that
decides boundary bits needs ~1e-5 accuracy (min |logit+logistic| = 8e-5)
and uses the split-bf16 scheme instead.
"""
import sys

sys.path.insert(0, "/opt/trn_rl_repo")

import numpy as np
import ml_dtypes
from contextlib import ExitStack

import concourse.bass as bass
import concourse.bacc as bacc
import concourse.tile as tile
from concourse import mybir
from concourse.masks import make_identity

F32 = mybir.dt.float32
F32R = mybir.dt.float32r
BF16 = mybir.dt.bfloat16
OP = mybir.AluOpType
ACT = mybir.ActivationFunctionType

B = 8
L_FULL = 2048
D_FULL = 1024
N_CORES = 8


def build(L=L_FULL, D=D_FULL):
    P = 128
    NLT = L // P          # 128-row l-tiles
    ND = D // P           # d-tiles
    CPT = L // P          # scan columns per partition (l = p*CPT + c)
    DC = min(512, D)      # matmul free-dim chunk
    NDC = D // DC
    LCH = min(512, L)     # l-chunk for MLP matmuls
    NLC = L // LCH
    WIN = 257             # selection window: prev tile (128) + cur (128) + null

    nc = bacc.Bacc("TRN2", target_bir_lowering=False, debug=False, num_devices=N_CORES)

    x_d = nc.dram_tensor("x", [L, D], F32, kind="ExternalInput").ap()
    xth_d = nc.dram_tensor("xth", [D, L], BF16, kind="ExternalInput").ap()
    xtl_d = nc.dram_tensor("xtl", [D, L], BF16, kind="ExternalInput").ap()
    noise_d = nc.dram_tensor("noise", [L], F32, kind="ExternalInput").ap()
    w1h_d = nc.dram_tensor("w1h", [D, D], BF16, kind="ExternalInput").ap()
    w1l_d = nc.dram_tensor("w1l", [D, D], BF16, kind="ExternalInput").ap()
    b1_d = nc.dram_tensor("b1", [D], F32, kind="ExternalInput").ap()
    w2_d = nc.dram_tensor("W2", [D], F32, kind="ExternalInput").ap()
    b2_d = nc.dram_tensor("b2", [1], F32, kind="ExternalInput").ap()
    null_d = nc.dram_tensor("null_group", [1, 1, D], F32, kind="ExternalInput").ap()
    up_d = nc.dram_tensor("up", [L, D], F32, kind="ExternalOutput").ap()

    with tile.TileContext(nc) as tc, ExitStack() as ctx:
        const = ctx.enter_context(tc.tile_pool(name="const", bufs=1))
        dram = ctx.enter_context(tc.tile_pool(name="dram", bufs=1, space="DRAM"))
        xpool = ctx.enter_context(tc.tile_pool(name="xp", bufs=3))
        xrp = ctx.enter_context(tc.tile_pool(name="xr", bufs=2))
        xtp = ctx.enter_context(tc.tile_pool(name="xtp", bufs=2))
        stage = ctx.enter_context(tc.tile_pool(name="stage", bufs=3))
        small = ctx.enter_context(tc.tile_pool(name="small", bufs=1))
        gpool = ctx.enter_context(tc.tile_pool(name="gp", bufs=2))
        logp = ctx.enter_context(tc.tile_pool(name="logp", bufs=2))
        psA = ctx.enter_context(tc.tile_pool(name="psA", bufs=2, space="PSUM"))
        psT = ctx.enter_context(tc.tile_pool(name="psT", bufs=2, space="PSUM"))
        psM = ctx.enter_context(tc.tile_pool(name="psM", bufs=2, space="PSUM"))

        # ---------------- DRAM scratch ----------------
        lg_scr = dram.tile([L, 1], F32)      # logits row bounce
        pe_scr = dram.tile([L, 1], F32)      # per-token segment-end row index
        ps_scr = dram.tile([L, 1], F32)      # per-token segment-start-minus-one row index
        r_scr = dram.tile([L, 1], F32)       # per-token reciprocal count

        # ---------------- constants ----------------
        const_dmas = []
        w1h_sb = const.tile([P, ND, D], BF16)
        const_dmas.append(nc.sync.dma_start(out=w1h_sb[:], in_=w1h_d.rearrange("(i p) n -> p i n", p=P)))
        w1l_sb = const.tile([P, ND, D], BF16)
        const_dmas.append(nc.sync.dma_start(out=w1l_sb[:], in_=w1l_d.rearrange("(i p) n -> p i n", p=P)))
        b1_sb = const.tile([P, ND], F32)
        const_dmas.append(nc.sync.dma_start(out=b1_sb[:], in_=b1_d.rearrange("(o p) -> p o", p=P)))
        w2_sb = const.tile([P, ND], F32)
        const_dmas.append(nc.sync.dma_start(out=w2_sb[:], in_=w2_d.rearrange("(o p) -> p o", p=P)))
        b2_sb = const.tile([1, 1], F32)
        const_dmas.append(nc.sync.dma_start(out=b2_sb[:], in_=b2_d.rearrange("(a b) -> a b", a=1)))
        null_sb = const.tile([1, D], F32)
        const_dmas.append(nc.sync.dma_start(out=null_sb[:], in_=null_d[0, 0, :].rearrange("(a d) -> a d", a=1)))

        # P_loc prefix tiles, SBUF-resident across the whole kernel.
        # fp32r: consumed (pre-rounded) by the final selection matmuls.
        pbig = const.tile([P, NLT, D], F32R)

        ident = const.tile([P, P], F32)
        make_identity(nc, ident[:])
        ident_r = const.tile([P, P], F32R)
        nc.vector.tensor_copy(out=ident_r[:], in_=ident[:])

        pio = const.tile([P, 1], F32)
        nc.gpsimd.iota(pio[:], pattern=[[0, 1]], base=0, channel_multiplier=1,
                       allow_small_or_imprecise_dtypes=True)
        fio = const.tile([P, P], F32)
        nc.gpsimd.iota(fio[:], pattern=[[1, P]], base=0, channel_multiplier=0,
                       allow_small_or_imprecise_dtypes=True)
        # ut[k, m] = 1 if k <= m   (inclusive prefix lhsT), fp32r for the
        # prefix matmuls
        ut = const.tile([P, P], F32R)
        nc.vector.tensor_scalar(out=ut[:], in0=fio[:], scalar1=pio[:], scalar2=None, op0=OP.is_ge)
        ones_col = const.tile([P, 1], F32)
        nc.vector.memset(ones_col[:], 1.0)
        ones_1x1 = const.tile([1, 1], F32)
        nc.vector.memset(ones_1x1[:], 1.0)
        zeros_cpt = const.tile([P, CPT], F32)
        nc.vector.memset(zeros_cpt[:], 0.0)
        zrow128 = const.tile([1, P], F32)
        nc.vector.memset(zrow128[:], 0.0)
        iotp1 = const.tile([P, CPT], F32)   # l + 1 (l = p*CPT + c), exact in f32
        nc.gpsimd.iota(iotp1[:], pattern=[[1, CPT]], base=1, channel_multiplier=CPT,
                       allow_small_or_imprecise_dtypes=True)
        # window iota: cols 0..255 hold 0..255 (local row index), col 256 is the
        # null slot (matched by clamped pe for null tokens)
        iwin = const.tile([P, WIN], F32)
        nc.gpsimd.iota(iwin[:], pattern=[[1, WIN]], base=0, channel_multiplier=0,
                       allow_small_or_imprecise_dtypes=True)
        # null row rounded to fp32r for the t=0 selection matmul
        null_r = const.tile([1, D], F32R)
        nc.vector.tensor_copy(out=null_r[:], in_=null_sb[:])

        # Collapse the fan of constant-load DMA lanes into one tick so later
        # matmuls don't exceed the per-instruction sync-wait slot limit.  The
        # barrier NOP itself is subject to the same limit, so first absorb the
        # DMA-lane semaphores into the SP clock with nops of <=4 deps each.
        from concourse.tile_rust import add_dep_helper as _adh
        for g in range(0, len(const_dmas), 4):
            spn = nc.sync.nop()
            for d in const_dmas[g:g + 4]:
                _adh(spn.ins, d.ins, sync=True, reason="const-lane coalesce")
        tc.strict_bb_all_engine_barrier()

        # ------- phases 1+2: per 512-token chunk: load, prefix, MLP -------
        for lc in range(NLC):
            lsl = slice(lc * LCH, (lc + 1) * LCH)
            # host-pretransposed hi/lo bf16 xT for this chunk
            xth_ch = xtp.tile([P, ND, LCH], BF16, tag="xTh")
            nc.sync.dma_start(out=xth_ch[:], in_=xth_d[:, lsl].rearrange("(i p) l -> p i l", p=P))
            xtl_ch = xtp.tile([P, ND, LCH], BF16, tag="xTl")
            nc.sync.dma_start(out=xtl_ch[:], in_=xtl_d[:, lsl].rearrange("(i p) l -> p i l", p=P))

            for ii in range(LCH // P):
                i = lc * (LCH // P) + ii
                x_t = xpool.tile([P, D], F32, tag="x")
                nc.sync.dma_start(out=x_t[:], in_=x_d[i * P:(i + 1) * P, :])
                # round to fp32r for the prefix matmul
                x_r = xrp.tile([P, D], F32R, tag="xr")
                nc.vector.tensor_copy(out=x_r[:], in_=x_t[:])

                # per-tile prefix (no cross-tile carry; folded into phase 4)
                psP = psA.tile([P, D], F32, tag="P")
                for dc in range(NDC):
                    sl = slice(dc * DC, (dc + 1) * DC)
                    nc.tensor.matmul(
                        psP[:, sl], lhsT=ut[:], rhs=x_r[:, sl],
                        start=True, stop=True,
                    )
                nc.vector.tensor_copy(out=pbig[:, i, :], in_=psP[:])

            # MLP for this l-chunk: 3-pass split-bf16
            logacc = logp.tile([P, LCH], F32, tag="logacc")
            for o in range(ND):
                psm = psM.tile([P, LCH], F32, tag="mlp")
                n_mm = ND * 3
                k = 0
                for i_ in range(ND):
                    for lhs_t, rhs_t in (
                        (w1h_sb, xth_ch), (w1l_sb, xth_ch), (w1h_sb, xtl_ch),
                    ):
                        nc.tensor.matmul(
                            psm[:],
                            lhsT=lhs_t[:, i_, o * P:(o + 1) * P],
                            rhs=rhs_t[:, i_, :],
                            start=(k == 0), stop=(k == n_mm - 1),
                        )
                        k += 1
                hT = stage.tile([P, LCH], F32, tag="hT")
                nc.scalar.activation(
                    out=hT[:], in_=psm[:], func=ACT.Relu,
                    bias=b1_sb[:, o:o + 1], scale=1.0,
                )
                if o == 0:
                    nc.vector.tensor_scalar(
                        out=logacc[:], in0=hT[:],
                        scalar1=w2_sb[:, o:o + 1], scalar2=None, op0=OP.mult,
                    )
                else:
                    nc.vector.scalar_tensor_tensor(
                        out=logacc[:], in0=hT[:], scalar=w2_sb[:, o:o + 1],
                        in1=logacc[:], op0=OP.mult, op1=OP.add,
                    )

            # logits partial for this chunk: partition-reduce + bias, to DRAM
            pslg = psM.tile([1, LCH], F32, tag="mlp")
            nc.tensor.matmul(pslg[:], lhsT=ones_col[:], rhs=logacc[:], start=True, stop=True)
            lg_ch = stage.tile([1, LCH], F32, tag="lgch")
            nc.scalar.activation(
                out=lg_ch[:], in_=pslg[:], func=ACT.Identity,
                bias=b2_sb[:, 0:1], scale=1.0,
            )
            nc.sync.dma_start(
                out=lg_scr[lsl, 0].rearrange("(a l) -> a l", a=1), in_=lg_ch[:]
            )

        # ---------------- phase 3: boundary bits, cumsum ----------------
        lg16 = small.tile([P, CPT], F32, tag="lg16")
        nc.sync.dma_start(out=lg16[:], in_=lg_scr[:, 0].rearrange("(p c) -> p c", c=CPT))

        nz16 = small.tile([P, CPT], F32, tag="nz")
        nc.sync.dma_start(out=nz16[:], in_=noise_d.rearrange("(p c) -> p c", c=CPT))

        lnu = small.tile([P, CPT], F32, tag="lnu")
        nc.scalar.activation(out=lnu[:], in_=nz16[:], func=ACT.Ln)
        om = small.tile([P, CPT], F32, tag="om")
        nc.vector.tensor_scalar(
            out=om[:], in0=nz16[:], scalar1=1.0, scalar2=-1.0,
            op0=OP.subtract, op1=OP.mult,
        )  # (u - 1) * -1 = 1 - u
        ln1m = small.tile([P, CPT], F32, tag="ln1m")
        nc.scalar.activation(out=ln1m[:], in_=om[:], func=ACT.Ln)
        tt = small.tile([P, CPT], F32, tag="tt")
        nc.vector.tensor_tensor(out=tt[:], in0=lnu[:], in1=ln1m[:], op=OP.subtract)
        nc.vector.tensor_tensor(out=tt[:], in0=tt[:], in1=lg16[:], op=OP.add)
        hard = small.tile([P, CPT], F32, tag="hard")
        nc.vector.tensor_scalar(out=hard[:], in0=tt[:], scalar1=0.0, scalar2=None, op0=OP.is_gt)

        # ---- prefix-max scans: lb_inc (last boundary <= l), scan2 (boundary before it)
        def cross_part_max_scan(inclusive, tagp):
            """Combine per-partition inclusive max-scans into a global scan.

            Returns a (P, CPT) tile where each row has been max-ed with the
            running max of all previous partitions' row-maxima.
            """
            # row maxima -> (1, P) via matmul with identity rhs
            ps_r = psM.tile([P, 512], F32, tag="mlp")
            nc.tensor.matmul(
                ps_r[0:1, 0:P], lhsT=inclusive[:, CPT - 1:CPT], rhs=ident[:],
                start=True, stop=True,
            )
            rowT = small.tile([1, P], F32, tag=tagp + "_rowT")
            nc.vector.tensor_copy(out=rowT[:], in_=ps_r[0:1, 0:P])
            # inclusive scan along the (1, P) row, then shift right one (exclusive)
            sc = small.tile([1, P], F32, tag=tagp + "_sc")
            nc.vector.tensor_tensor_scan(
                out=sc[:], data0=rowT[:], data1=zrow128[:],
                initial=-1.0, op0=OP.max, op1=OP.add,
            )
            exc = small.tile([1, P], F32, tag=tagp + "_exc")
            nc.vector.memset(exc[0:1, 0:1], -1.0)
            nc.vector.tensor_copy(out=exc[0:1, 1:P], in_=sc[0:1, 0:P - 1])
            # back to (P, 1) via rank-1 matmul with ones (1,1)
            ps_b = psM.tile([P, 512], F32, tag="mlp")
            nc.tensor.matmul(
                ps_b[:, 0:1], lhsT=exc[:], rhs=ones_1x1[:], start=True, stop=True,
            )
            offm = small.tile([P, 1], F32, tag=tagp + "_offm")
            nc.vector.tensor_copy(out=offm[:], in_=ps_b[:, 0:1])
            out_t = small.tile([P, CPT], F32, tag=tagp + "_out")
            nc.vector.tensor_scalar(
                out=out_t[:], in0=inclusive[:], scalar1=offm[:], scalar2=None, op0=OP.max,
            )
            return out_t, offm

        # mi = hard ? l : -1  == (l+1)*hard - 1
        mi = small.tile([P, CPT], F32, tag="mi")
        nc.vector.tensor_tensor(out=mi[:], in0=iotp1[:], in1=hard[:], op=OP.mult)
        nc.vector.tensor_scalar(out=mi[:], in0=mi[:], scalar1=-1.0, scalar2=None, op0=OP.add)
        s1l = small.tile([P, CPT], F32, tag="s1l")
        nc.vector.tensor_tensor_scan(
            out=s1l[:], data0=mi[:], data1=zeros_cpt[:],
            initial=-1.0, op0=OP.max, op1=OP.add,
        )
        lb_inc, offm1 = cross_part_max_scan(s1l, "s1")

        # lbm1[l] = lb_inc[l-1] (token shift; layout l = p*CPT + c).
        # Column 0 of partition p is lb_inc at the end of partition p-1,
        # which is exactly the exclusive cross-partition max offm1.
        lbm1 = small.tile([P, CPT], F32, tag="lbm1")
        nc.vector.tensor_copy(out=lbm1[:, 0:1], in_=offm1[:])
        nc.vector.tensor_copy(out=lbm1[:, 1:CPT], in_=lb_inc[:, 0:CPT - 1])
        # mi2 = hard ? lbm1 : -1 == (lbm1+1)*hard - 1
        mi2 = small.tile([P, CPT], F32, tag="mi2")
        nc.vector.tensor_scalar(out=mi2[:], in0=lbm1[:], scalar1=1.0, scalar2=None, op0=OP.add)
        nc.vector.tensor_tensor(out=mi2[:], in0=mi2[:], in1=hard[:], op=OP.mult)
        nc.vector.tensor_scalar(out=mi2[:], in0=mi2[:], scalar1=-1.0, scalar2=None, op0=OP.add)
        s2l = small.tile([P, CPT], F32, tag="s2l")
        nc.vector.tensor_tensor_scan(
            out=s2l[:], data0=mi2[:], data1=zeros_cpt[:],
            initial=-1.0, op0=OP.max, op1=OP.add,
        )
        pb, _ = cross_part_max_scan(s2l, "s2")

        # cnt = lb_inc - pb ;  r = 1/(cnt + 1e-9), forced to 1.0 for null tokens
        cnt = small.tile([P, CPT], F32, tag="cnt")
        nc.vector.tensor_tensor(out=cnt[:], in0=lb_inc[:], in1=pb[:], op=OP.subtract)
        nc.vector.tensor_scalar(out=cnt[:], in0=cnt[:], scalar1=1e-9, scalar2=None, op0=OP.add)
        r_tok = small.tile([P, CPT], F32, tag="r_tok")
        nc.vector.reciprocal(out=r_tok[:], in_=cnt[:])
        mask0 = small.tile([P, CPT], F32, tag="mask0")
        nc.vector.tensor_scalar(out=mask0[:], in0=lb_inc[:], scalar1=-0.5, scalar2=None, op0=OP.is_gt)
        # r_tok = (r_tok - 1)*mask0 + 1
        nc.vector.tensor_scalar(out=r_tok[:], in0=r_tok[:], scalar1=-1.0, scalar2=None, op0=OP.add)
        nc.vector.tensor_tensor(out=r_tok[:], in0=r_tok[:], in1=mask0[:], op=OP.mult)
        nc.vector.tensor_scalar(out=r_tok[:], in0=r_tok[:], scalar1=1.0, scalar2=None, op0=OP.add)
        # pe = mask0 ? lb_inc : L+1 (null slot)   == (lb_inc - (L+1))*mask0 + (L+1)
        pe_t = small.tile([P, CPT], F32, tag="pe_t")
        nc.vector.tensor_scalar(out=pe_t[:], in0=lb_inc[:], scalar1=-float(L + 1), scalar2=None, op0=OP.add)
        nc.vector.tensor_tensor(out=pe_t[:], in0=pe_t[:], in1=mask0[:], op=OP.mult)
        nc.vector.tensor_scalar(out=pe_t[:], in0=pe_t[:], scalar1=float(L + 1), scalar2=None, op0=OP.add)
        # ps = pb >= 0 ? pb : L (zero contribution)  == (pb - L)*mask2 + L
        mask2 = small.tile([P, CPT], F32, tag="mask2")
        nc.vector.tensor_scalar(out=mask2[:], in0=pb[:], scalar1=-0.5, scalar2=None, op0=OP.is_gt)
        ps_t2 = small.tile([P, CPT], F32, tag="ps_t2")
        nc.vector.tensor_scalar(out=ps_t2[:], in0=pb[:], scalar1=-float(L), scalar2=None, op0=OP.add)
        nc.vector.tensor_tensor(out=ps_t2[:], in0=ps_t2[:], in1=mask2[:], op=OP.mult)
        nc.vector.tensor_scalar(out=ps_t2[:], in0=ps_t2[:], scalar1=float(L), scalar2=None, op0=OP.add)

        # layout bounce (p*CPT+c) -> (128t+p) chunked, all f32
        nc.sync.dma_start(out=pe_scr[:, 0].rearrange("(p c) -> p c", c=CPT), in_=pe_t[:])
        nc.sync.dma_start(out=ps_scr[:, 0].rearrange("(p c) -> p c", c=CPT), in_=ps_t2[:])
        nc.sync.dma_start(out=r_scr[:, 0].rearrange("(p c) -> p c", c=CPT), in_=r_tok[:])
        pe2 = small.tile([P, NLT], F32, tag="pe2")
        nc.sync.dma_start(out=pe2[:], in_=pe_scr[:, 0].rearrange("(t p) -> p t", p=P))
        ps2 = small.tile([P, NLT], F32, tag="ps2")
        nc.sync.dma_start(out=ps2[:], in_=ps_scr[:, 0].rearrange("(t p) -> p t", p=P))
        r2 = small.tile([P, NLT], F32, tag="r2")
        nc.sync.dma_start(out=r2[:], in_=r_scr[:, 0].rearrange("(t p) -> p t", p=P))

        # ---- final: per 128-token tile, one-hot selection matmul over the
        # window [prev tile rows | cur tile rows | null slot] of P_loc ----
        for t in range(NLT):
            base = (t - 1) * P  # global row index of window col 0

            # local window indices; clamp pe to the null slot (256).
            # pe in {0..L-1} U {L+1}; normal pe-base <= 255, null >= 257 -> 256.
            pel = gpool.tile([P, 1], F32, tag="pel")
            nc.vector.tensor_scalar(
                out=pel[:], in0=pe2[:, t:t + 1], scalar1=-float(base),
                scalar2=float(WIN - 1), op0=OP.add, op1=OP.min,
            )
            # ps in {0..L-1} U {L}; ps=L (zero row) must match nothing: no clamp
            psl = gpool.tile([P, 1], F32, tag="psl")
            nc.vector.tensor_scalar(
                out=psl[:], in0=ps2[:, t:t + 1], scalar1=-float(base),
                scalar2=None, op0=OP.add,
            )

            ct = gpool.tile([P, WIN], F32R, tag="ct")
            ct2 = gpool.tile([P, WIN], F32R, tag="ct2")
            nc.vector.tensor_scalar(out=ct[:], in0=iwin[:], scalar1=pel[:], scalar2=None, op0=OP.is_equal)
            nc.vector.tensor_scalar(out=ct2[:], in0=iwin[:], scalar1=psl[:], scalar2=None, op0=OP.is_equal)
            nc.vector.tensor_tensor(out=ct[:], in0=ct[:], in1=ct2[:], op=OP.subtract)

            if t > 0:
                # cross-tile carry: pe in cur tile & ps in prev tile ->
                # +1 on the last row of the prev tile (adds its row-sum,
                # i.e. the inter-tile offset difference)
                crA = gpool.tile([P, 1], F32R, tag="crA")
                nc.vector.tensor_scalar(out=crA[:], in0=pe2[:, t:t + 1],
                                        scalar1=float(base + P), scalar2=None, op0=OP.is_ge)
                crB = gpool.tile([P, 1], F32R, tag="crB")
                nc.vector.tensor_scalar(out=crB[:], in0=ps2[:, t:t + 1],
                                        scalar1=float(base + P), scalar2=None, op0=OP.is_lt)
                nc.vector.tensor_tensor(out=crA[:], in0=crA[:], in1=crB[:], op=OP.mult)
                nc.vector.tensor_tensor(out=ct[:, P - 1:P], in0=ct[:, P - 1:P], in1=crA[:], op=OP.add)

            # transpose CT -> C chunks (lhsT for the selection matmuls)
            ps_c = psT.tile([P, 512], F32R, tag="trR")
            if t > 0:
                nc.tensor.transpose(out=ps_c[:, 0:P], in_=ct[:, 0:P], identity=ident_r[:])
            nc.tensor.transpose(out=ps_c[:, P:2 * P], in_=ct[:, P:2 * P], identity=ident_r[:])
            if t == 0:
                nc.tensor.transpose(out=ps_c[0:1, 2 * P:3 * P], in_=ct[:, 2 * P:2 * P + 1],
                                    identity=ident_r[:])
            c_sb = gpool.tile([P, 3 * P], F32R, tag="c_sb")
            if t > 0:
                nc.vector.tensor_copy(out=c_sb[:, 0:2 * P], in_=ps_c[:, 0:2 * P])
            else:
                nc.vector.tensor_copy(out=c_sb[:, P:2 * P], in_=ps_c[:, P:2 * P])
                nc.vector.tensor_copy(out=c_sb[0:1, 2 * P:3 * P], in_=ps_c[0:1, 2 * P:3 * P])

            psO = psA.tile([P, D], F32, tag="P")
            for dc in range(NDC):
                sl = slice(dc * DC, (dc + 1) * DC)
                if t > 0:
                    nc.tensor.matmul(psO[:, sl], lhsT=c_sb[:, 0:P],
                                     rhs=pbig[:, t - 1, sl],
                                     start=True, stop=False)
                    nc.tensor.matmul(psO[:, sl], lhsT=c_sb[:, P:2 * P],
                                     rhs=pbig[:, t, sl],
                                     start=False, stop=True)
                else:
                    nc.tensor.matmul(psO[:, sl], lhsT=c_sb[:, P:2 * P],
                                     rhs=pbig[:, t, sl],
                                     start=True, stop=False)
                    nc.tensor.matmul(psO[:, sl], lhsT=c_sb[0:1, 2 * P:3 * P],
                                     rhs=null_r[0:1, sl],
                                     start=False, stop=True)

            upt = stage.tile([P, D], F32, tag="up")
            nc.scalar.activation(out=upt[:], in_=psO[:], func=ACT.Copy,
                                 bias=0.0, scale=r2[:, t:t + 1])
            nc.sync.dma_start(out=up_d[t * P:(t + 1) * P, :], in_=upt[:])

    nc.compile()
    return nc


_nc_cache = {}


def _get_nc(L, D):
    key = (L, D)
    if key not in _nc_cache:
        _nc_cache[key] = build(L, D)
    return _nc_cache[key]


def make_in_maps(inputs, n_cores=N_CORES):
    bf16 = ml_dtypes.bfloat16
    x = np.ascontiguousarray(np.asarray(inputs["x"], dtype=np.float32))
    noise = np.ascontiguousarray(np.asarray(inputs["noise"], dtype=np.float32))
    w1 = np.ascontiguousarray(np.asarray(inputs["W1"], dtype=np.float32))
    w1h = w1.astype(bf16)
    w1l = (w1 - w1h.astype(np.float32)).astype(bf16)
    shared = {
        "w1h": np.ascontiguousarray(w1h),
        "w1l": np.ascontiguousarray(w1l),
        "b1": np.ascontiguousarray(np.asarray(inputs["b1"], dtype=np.float32)),
        "W2": np.ascontiguousarray(np.asarray(inputs["W2"], dtype=np.float32)),
        "b2": np.ascontiguousarray(np.asarray(inputs["b2"], dtype=np.float32)),
        "null_group": np.ascontiguousarray(np.asarray(inputs["null_group"], dtype=np.float32)),
    }
    maps = []
    for c in range(n_cores):
        xT = np.ascontiguousarray(x[c].T)           # [D, L] fp32
        xth = xT.astype(bf16)
        xtl = (xT - xth.astype(np.float32)).astype(bf16)
        maps.append(dict(
            shared, x=x[c], noise=noise[c],
            xth=np.ascontiguousarray(xth), xtl=np.ascontiguousarray(xtl),
        ))
    return maps


def kernel(**inputs):
    from concourse.bass_utils import run_bass_kernel_spmd

    x = np.asarray(inputs["x"])
    b, L, D = x.shape
    assert b == N_CORES
    nc = _get_nc(L, D)
    in_maps = make_in_maps(inputs)
    res = run_bass_kernel_spmd(nc, in_maps, core_ids=list(range(N_CORES)))
    out = np.stack([res.results[c]["up"] for c in range(N_CORES)], axis=0)
    return out.astype(np.float32)


# revision 25
# speedup vs baseline: 1.8104x; 1.0894x over previous
"""Trainium2 Bass kernel for the Nawrot downsampler-upsampler module.

Per-core (data-parallel over batch, 1 example per NeuronCore):
  1. PE per-tile prefix sums of x along L (triangular fp32r matmul, NO
     cross-tile carry: the carry is folded into the final selection matmul
     as a +1 coefficient on the last row of the previous tile) -> P_loc
     kept in SBUF.
  2. MLP relu(x@W1+b1) via 3-pass split-bf16 matmuls (x = xh+xl, W1 =
     wh+wl; xh*wh + xh*wl + xl*wh reproduces fp32 to ~2^-16) over
     host-pretransposed xT; logits via ones-matmul partition reduce.
  3. boundary bits from logits + logistic noise; segment scans via
     tensor_tensor_scan + cross-partition max combine.
  4. final: per 128-token tile, build a +-1 one-hot coefficient matrix
     (segment end minus segment start-1, window = prev tile + cur tile +
     null slot) and contract it against the SBUF-resident P_loc tiles with
     fp32r matmuls; scale by 1/count on the scalar engine.

fp32r (4-byte, ~12 mantissa bits, 1 cycle/row vs fp32's 4) is used where
the ~2e-4 rounding is harmless (segment averages); the logits path that
decides boundary bits needs ~1e-5 accuracy (min |logit+logistic| = 8e-5)
and uses the split-bf16 scheme instead.
"""
import sys

sys.path.insert(0, "/opt/trn_rl_repo")

import numpy as np
import ml_dtypes
from contextlib import ExitStack

import concourse.bass as bass
import concourse.bacc as bacc
import concourse.tile as tile
from concourse import mybir
from concourse.masks import make_identity

F32 = mybir.dt.float32
F32R = mybir.dt.float32r
BF16 = mybir.dt.bfloat16
OP = mybir.AluOpType
ACT = mybir.ActivationFunctionType

B = 8
L_FULL = 2048
D_FULL = 1024
N_CORES = 8


def build(L=L_FULL, D=D_FULL, dbg=False):
    P = 128
    NLT = L // P          # 128-row l-tiles
    ND = D // P           # d-tiles
    CPT = L // P          # scan columns per partition (l = p*CPT + c)
    DC = min(512, D)      # matmul free-dim chunk
    NDC = D // DC
    LCH = min(512, L)     # l-chunk for MLP matmuls
    NLC = L // LCH
    WIN = 257             # selection window: prev tile (128) + cur (128) + null
    CAP = 128             # fixup slots for near-threshold logit recompute
    TAU = 0.02            # |logit+logistic| band needing exact recompute

    nc = bacc.Bacc("TRN2", target_bir_lowering=False, debug=False, num_devices=N_CORES)

    x_d = nc.dram_tensor("x", [L, D], F32, kind="ExternalInput").ap()
    xth_d = nc.dram_tensor("xth", [D, L], BF16, kind="ExternalInput").ap()
    noise_d = nc.dram_tensor("noise", [L], F32, kind="ExternalInput").ap()
    w1h_d = nc.dram_tensor("w1h", [D, D], BF16, kind="ExternalInput").ap()
    w1l_d = nc.dram_tensor("w1l", [D, D], BF16, kind="ExternalInput").ap()
    b1_d = nc.dram_tensor("b1", [D], F32, kind="ExternalInput").ap()
    w2_d = nc.dram_tensor("W2", [D], F32, kind="ExternalInput").ap()
    b2_d = nc.dram_tensor("b2", [1], F32, kind="ExternalInput").ap()
    null_d = nc.dram_tensor("null_group", [1, 1, D], F32, kind="ExternalInput").ap()
    up_d = nc.dram_tensor("up", [L, D], F32, kind="ExternalOutput").ap()
    if dbg:
        dbg_idx = nc.dram_tensor("dbg_idx", [128, 1], F32, kind="ExternalOutput").ap()
        dbg_lgcol = nc.dram_tensor("dbg_lgcol", [128, 1], F32, kind="ExternalOutput").ap()
        dbg_lg16b = nc.dram_tensor("dbg_lg16b", [128, 16], F32, kind="ExternalOutput").ap()
        dbg_zap = nc.dram_tensor("dbg_zap", [128, 16], F32, kind="ExternalOutput").ap()
        dbg_hard = nc.dram_tensor("dbg_hard", [128, 16], F32, kind="ExternalOutput").ap()
        dbg_oi = nc.dram_tensor("dbg_oi", [128, 16], mybir.dt.int32, kind="ExternalOutput").ap()

    with tile.TileContext(nc) as tc, ExitStack() as ctx:
        const = ctx.enter_context(tc.tile_pool(name="const", bufs=1))
        dram = ctx.enter_context(tc.tile_pool(name="dram", bufs=1, space="DRAM"))
        xpool = ctx.enter_context(tc.tile_pool(name="xp", bufs=3))
        xrp = ctx.enter_context(tc.tile_pool(name="xr", bufs=2))
        xtp = ctx.enter_context(tc.tile_pool(name="xtp", bufs=2))
        stage = ctx.enter_context(tc.tile_pool(name="stage", bufs=3))
        small = ctx.enter_context(tc.tile_pool(name="small", bufs=1))
        gpool = ctx.enter_context(tc.tile_pool(name="gp", bufs=2))
        logp = ctx.enter_context(tc.tile_pool(name="logp", bufs=2))
        psA = ctx.enter_context(tc.tile_pool(name="psA", bufs=2, space="PSUM"))
        psT = ctx.enter_context(tc.tile_pool(name="psT", bufs=2, space="PSUM"))
        psM = ctx.enter_context(tc.tile_pool(name="psM", bufs=2, space="PSUM"))

        # ---------------- DRAM scratch ----------------
        lg_scr = dram.tile([L, 1], F32)      # logits row bounce
        pe_scr = dram.tile([L, 1], F32)      # per-token segment-end row index
        ps_scr = dram.tile([L, 1], F32)      # per-token segment-start-minus-one row index
        r_scr = dram.tile([L, 1], F32)       # per-token reciprocal count

        # ---------------- constants ----------------
        # (w1h/w1l DMAs are issued after the all-engine barrier so the x /
        # prefix pipeline is not gated on the 4 MB weight load)
        const_dmas = []
        w1h_sb = const.tile([P, ND, D], BF16)
        w1l_sb = const.tile([P, ND, D], BF16)
        b1_sb = const.tile([P, ND], F32)
        const_dmas.append(nc.sync.dma_start(out=b1_sb[:], in_=b1_d.rearrange("(o p) -> p o", p=P)))
        w2_sb = const.tile([P, ND], F32)
        const_dmas.append(nc.sync.dma_start(out=w2_sb[:], in_=w2_d.rearrange("(o p) -> p o", p=P)))
        b2_sb = const.tile([1, 1], F32)
        const_dmas.append(nc.sync.dma_start(out=b2_sb[:], in_=b2_d.rearrange("(a b) -> a b", a=1)))
        null_sb = const.tile([1, D], F32)
        const_dmas.append(nc.sync.dma_start(out=null_sb[:], in_=null_d[0, 0, :].rearrange("(a d) -> a d", a=1)))

        # P_loc prefix tiles, SBUF-resident across the whole kernel.
        # fp32r: consumed (pre-rounded) by the final selection matmuls.
        pbig = const.tile([P, NLT, D], F32R)

        ident = const.tile([P, P], F32)
        make_identity(nc, ident[:])
        ident_r = const.tile([P, P], F32R)
        nc.vector.tensor_copy(out=ident_r[:], in_=ident[:])

        pio = const.tile([P, 1], F32)
        nc.gpsimd.iota(pio[:], pattern=[[0, 1]], base=0, channel_multiplier=1,
                       allow_small_or_imprecise_dtypes=True)
        fio = const.tile([P, P], F32)
        nc.gpsimd.iota(fio[:], pattern=[[1, P]], base=0, channel_multiplier=0,
                       allow_small_or_imprecise_dtypes=True)
        # ut[k, m] = 1 if k <= m   (inclusive prefix lhsT), fp32r for the
        # prefix matmuls
        ut = const.tile([P, P], F32R)
        nc.vector.tensor_scalar(out=ut[:], in0=fio[:], scalar1=pio[:], scalar2=None, op0=OP.is_ge)
        ones_col = const.tile([P, 1], F32)
        nc.vector.memset(ones_col[:], 1.0)
        ones_1x1 = const.tile([1, 1], F32)
        nc.vector.memset(ones_1x1[:], 1.0)
        zeros_cpt = const.tile([P, CPT], F32)
        nc.vector.memset(zeros_cpt[:], 0.0)
        zrow128 = const.tile([1, P], F32)
        nc.vector.memset(zrow128[:], 0.0)
        iotp1 = const.tile([P, CPT], F32)   # l + 1 (l = p*CPT + c), exact in f32
        nc.gpsimd.iota(iotp1[:], pattern=[[1, CPT]], base=1, channel_multiplier=CPT,
                       allow_small_or_imprecise_dtypes=True)
        # window iota: cols 0..255 hold 0..255 (local row index), col 256 is the
        # null slot (matched by clamped pe for null tokens)
        iwin = const.tile([P, WIN], F32)
        nc.gpsimd.iota(iwin[:], pattern=[[1, WIN]], base=0, channel_multiplier=0,
                       allow_small_or_imprecise_dtypes=True)
        # null row rounded to fp32r for the t=0 selection matmul
        null_r = const.tile([1, D], F32R)
        nc.vector.tensor_copy(out=null_r[:], in_=null_sb[:])

        # Collapse the fan of constant-load DMA lanes into one tick so later
        # matmuls don't exceed the per-instruction sync-wait slot limit.  The
        # barrier NOP itself is subject to the same limit, so first absorb the
        # DMA-lane semaphores into the SP clock with nops of <=4 deps each.
        from concourse.tile_rust import add_dep_helper as _adh
        for g in range(0, len(const_dmas), 4):
            spn = nc.sync.nop()
            for d in const_dmas[g:g + 4]:
                _adh(spn.ins, d.ins, sync=True, reason="const-lane coalesce")
        tc.strict_bb_all_engine_barrier()

        nc.sync.dma_start(out=w1h_sb[:], in_=w1h_d.rearrange("(i p) n -> p i n", p=P))
        nc.sync.dma_start(out=w1l_sb[:], in_=w1l_d.rearrange("(i p) n -> p i n", p=P))

        # shared bounds-check registers for the fixup's indirect DMAs
        bc_cap = nc.gpsimd.to_reg(CAP - 1)
        bc_row = nc.gpsimd.to_reg(L - 1)

        # ------- phases 1+2: per 512-token chunk: load, prefix, MLP -------
        for lc in range(NLC):
            lsl = slice(lc * LCH, (lc + 1) * LCH)
            # host-pretransposed bf16 xT (hi part) for this chunk
            xth_ch = xtp.tile([P, ND, LCH], BF16, tag="xTh")
            nc.sync.dma_start(out=xth_ch[:], in_=xth_d[:, lsl].rearrange("(i p) l -> p i l", p=P))

            for ii in range(LCH // P):
                i = lc * (LCH // P) + ii
                x_t = xpool.tile([P, D], F32, tag="x")
                nc.sync.dma_start(out=x_t[:], in_=x_d[i * P:(i + 1) * P, :])
                # round to fp32r for the prefix matmul
                x_r = xrp.tile([P, D], F32R, tag="xr")
                nc.vector.tensor_copy(out=x_r[:], in_=x_t[:])

                # per-tile prefix (no cross-tile carry; folded into phase 4)
                psP = psA.tile([P, D], F32, tag="P")
                for dc in range(NDC):
                    sl = slice(dc * DC, (dc + 1) * DC)
                    nc.tensor.matmul(
                        psP[:, sl], lhsT=ut[:], rhs=x_r[:, sl],
                        start=True, stop=True,
                    )
                nc.vector.tensor_copy(out=pbig[:, i, :], in_=psP[:])

            # MLP pass A for this l-chunk: single bf16 pass (xh*wh).  Logit
            # error <= ~7e-3; tokens within TAU of the boundary threshold are
            # recomputed exactly in the fixup phase below.
            logacc = logp.tile([P, LCH], F32, tag="logacc")
            for o in range(ND):
                psm = psM.tile([P, LCH], F32, tag="mlp")
                for i_ in range(ND):
                    nc.tensor.matmul(
                        psm[:],
                        lhsT=w1h_sb[:, i_, o * P:(o + 1) * P],
                        rhs=xth_ch[:, i_, :],
                        start=(i_ == 0), stop=(i_ == ND - 1),
                    )
                hT = stage.tile([P, LCH], F32, tag="hT")
                nc.scalar.activation(
                    out=hT[:], in_=psm[:], func=ACT.Relu,
                    bias=b1_sb[:, o:o + 1], scale=1.0,
                )
                if o == 0:
                    nc.vector.tensor_scalar(
                        out=logacc[:], in0=hT[:],
                        scalar1=w2_sb[:, o:o + 1], scalar2=None, op0=OP.mult,
                    )
                else:
                    nc.vector.scalar_tensor_tensor(
                        out=logacc[:], in0=hT[:], scalar=w2_sb[:, o:o + 1],
                        in1=logacc[:], op0=OP.mult, op1=OP.add,
                    )

            # logits partial for this chunk: partition-reduce + bias, to DRAM
            pslg = psM.tile([1, LCH], F32, tag="mlp")
            nc.tensor.matmul(pslg[:], lhsT=ones_col[:], rhs=logacc[:], start=True, stop=True)
            lg_ch = stage.tile([1, LCH], F32, tag="lgch")
            nc.scalar.activation(
                out=lg_ch[:], in_=pslg[:], func=ACT.Identity,
                bias=b2_sb[:, 0:1], scale=1.0,
            )
            nc.sync.dma_start(
                out=lg_scr[lsl, 0].rearrange("(a l) -> a l", a=1), in_=lg_ch[:]
            )

        # ---------------- phase 3: boundary bits, cumsum ----------------
        lg16 = small.tile([P, CPT], F32, tag="lg16")
        nc.sync.dma_start(out=lg16[:], in_=lg_scr[:, 0].rearrange("(p c) -> p c", c=CPT))

        nz16 = small.tile([P, CPT], F32, tag="nz")
        nc.sync.dma_start(out=nz16[:], in_=noise_d.rearrange("(p c) -> p c", c=CPT))

        lnu = small.tile([P, CPT], F32, tag="lnu")
        nc.scalar.activation(out=lnu[:], in_=nz16[:], func=ACT.Ln)
        om = small.tile([P, CPT], F32, tag="om")
        nc.vector.tensor_scalar(
            out=om[:], in0=nz16[:], scalar1=1.0, scalar2=-1.0,
            op0=OP.subtract, op1=OP.mult,
        )  # (u - 1) * -1 = 1 - u
        ln1m = small.tile([P, CPT], F32, tag="ln1m")
        nc.scalar.activation(out=ln1m[:], in_=om[:], func=ACT.Ln)
        logi = small.tile([P, CPT], F32, tag="logi")
        nc.vector.tensor_tensor(out=logi[:], in0=lnu[:], in1=ln1m[:], op=OP.subtract)
        zap = small.tile([P, CPT], F32, tag="zap")
        nc.vector.tensor_tensor(out=zap[:], in0=logi[:], in1=lg16[:], op=OP.add)

        # ---- logit fixup: exactly recompute tokens with |z| < TAU ----
        # candidate mask
        msq = small.tile([P, CPT], F32, tag="msq")
        nc.vector.tensor_tensor(out=msq[:], in0=zap[:], in1=zap[:], op=OP.mult)
        mca = small.tile([P, CPT], F32, tag="mca")
        nc.vector.tensor_scalar(out=mca[:], in0=msq[:], scalar1=TAU * TAU, scalar2=None, op0=OP.is_lt)
        # global exclusive rank of each candidate (token order l = p*CPT + c)
        minc = small.tile([P, CPT], F32, tag="minc")
        nc.vector.tensor_tensor_scan(
            out=minc[:], data0=mca[:], data1=zeros_cpt[:],
            initial=0.0, op0=OP.add, op1=OP.add,
        )
        ps_r2 = psM.tile([P, 512], F32, tag="mlp")
        nc.tensor.matmul(ps_r2[0:1, 0:P], lhsT=minc[:, CPT - 1:CPT], rhs=ident[:],
                         start=True, stop=True)
        rowT2 = small.tile([1, P], F32, tag="fx_rowT")
        nc.vector.tensor_copy(out=rowT2[:], in_=ps_r2[0:1, 0:P])
        sc2t = small.tile([1, P], F32, tag="fx_sc")
        nc.vector.tensor_tensor_scan(
            out=sc2t[:], data0=rowT2[:], data1=zrow128[:],
            initial=0.0, op0=OP.add, op1=OP.add,
        )
        exc2 = small.tile([1, P], F32, tag="fx_exc")
        nc.vector.memset(exc2[0:1, 0:1], 0.0)
        nc.vector.tensor_copy(out=exc2[0:1, 1:P], in_=sc2t[0:1, 0:P - 1])
        ps_b2 = psM.tile([P, 512], F32, tag="mlp")
        nc.tensor.matmul(ps_b2[:, 0:1], lhsT=exc2[:], rhs=ones_1x1[:], start=True, stop=True)
        offc = small.tile([P, 1], F32, tag="fx_offc")
        nc.vector.tensor_copy(out=offc[:], in_=ps_b2[:, 0:1])
        rank = small.tile([P, CPT], F32, tag="rank")
        nc.vector.tensor_scalar(out=rank[:], in0=minc[:], scalar1=offc[:], scalar2=None, op0=OP.add)
        nc.vector.tensor_tensor(out=rank[:], in0=rank[:], in1=mca[:], op=OP.subtract)
        candl = small.tile([P, CPT], F32, tag="candl")
        nc.vector.tensor_scalar(out=candl[:], in0=iotp1[:], scalar1=-1.0, scalar2=None, op0=OP.add)

        # slot-selection one-hots: sel[p, c, s] = (rank[p,c]==s) * mca[p,c].
        # Slot index of every candidate in [0, CAP); all-zero columns for
        # unused slots (those slots resolve to token 0, recomputed harmlessly).
        sel = small.tile([P, CPT, P], F32, tag="sel")
        for c in range(CPT):
            nc.vector.tensor_scalar(
                out=sel[:, c, :], in0=fio[:], scalar1=rank[:, c:c + 1],
                scalar2=mca[:, c:c + 1], op0=OP.is_equal, op1=OP.mult,
            )
        # idx[s] = sum_{p,c} sel[p,c,s] * token_index[p,c]  (PE contraction)
        ps_idx = psM.tile([P, 512], F32, tag="mlp")
        for c in range(CPT):
            nc.tensor.matmul(
                ps_idx[:, 0:1], lhsT=sel[:, c, :], rhs=candl[:, c:c + 1],
                start=(c == 0), stop=(c == CPT - 1),
            )
        idxf = small.tile([CAP, 1], F32, tag="idxf")
        nc.vector.tensor_copy(out=idxf[:], in_=ps_idx[:, 0:1])
        idxi = small.tile([CAP, 1], mybir.dt.int32, tag="idxi")
        nc.vector.tensor_copy(out=idxi[:], in_=idxf[:])

        # gather candidate token rows of x, transpose, split hi/lo bf16
        xcand = small.tile([P, D], F32, tag="xcand")
        nc.gpsimd.indirect_dma_start(
            out=xcand[:], out_offset=None, in_=x_d,
            in_offset=bass.IndirectOffsetOnAxis(ap=idxi[:], axis=0),
            bounds_check=bc_row, oob_is_err=False,
        )
        xtc_h = small.tile([P, ND, P], BF16, tag="xtc_h")
        xtc_l = small.tile([P, ND, P], BF16, tag="xtc_l")
        for g in range(2):
            psc_t = psM.tile([P, 512], F32, tag="mlp")
            for jj in range(4):
                j = g * 4 + jj
                nc.tensor.transpose(
                    out=psc_t[:, jj * P:(jj + 1) * P],
                    in_=xcand[:, j * P:(j + 1) * P],
                    identity=ident[:],
                )
            pview = psc_t[:, 0:512].rearrange("p (j q) -> p j q", q=P)
            nc.vector.tensor_copy(out=xtc_h[:, g * 4:g * 4 + 4, :], in_=pview)
            tmp32 = small.tile([P, 512], F32, tag="tmp32")
            tview = tmp32[:, 0:512].rearrange("p (j q) -> p j q", q=P)
            nc.vector.tensor_copy(out=tview, in_=xtc_h[:, g * 4:g * 4 + 4, :])
            nc.vector.tensor_tensor(out=xtc_l[:, g * 4:g * 4 + 4, :], in0=pview,
                                    in1=tview, op=OP.subtract)

        # exact 3-pass MLP on the candidate columns
        logacc_c = small.tile([P, P], F32, tag="logc")
        for o in range(ND):
            psc2 = psM.tile([P, 512], F32, tag="mlp")
            n_mm = ND * 3
            k = 0
            for i_ in range(ND):
                for lhs_t, rhs_t in (
                    (w1h_sb, xtc_h), (w1l_sb, xtc_h), (w1h_sb, xtc_l),
                ):
                    nc.tensor.matmul(
                        psc2[:, 0:P],
                        lhsT=lhs_t[:, i_, o * P:(o + 1) * P],
                        rhs=rhs_t[:, i_, :],
                        start=(k == 0), stop=(k == n_mm - 1),
                    )
                    k += 1
            hTc = small.tile([P, P], F32, tag="hTc")
            nc.scalar.activation(
                out=hTc[:], in_=psc2[:, 0:P], func=ACT.Relu,
                bias=b1_sb[:, o:o + 1], scale=1.0,
            )
            if o == 0:
                nc.vector.tensor_scalar(
                    out=logacc_c[:], in0=hTc[:],
                    scalar1=w2_sb[:, o:o + 1], scalar2=None, op0=OP.mult,
                )
            else:
                nc.vector.scalar_tensor_tensor(
                    out=logacc_c[:], in0=hTc[:], scalar=w2_sb[:, o:o + 1],
                    in1=logacc_c[:], op0=OP.mult, op1=OP.add,
                )
        pslgc = psM.tile([P, 512], F32, tag="mlp")
        nc.tensor.matmul(pslgc[0:1, 0:P], lhsT=ones_col[:], rhs=logacc_c[:],
                         start=True, stop=True)
        lgc = small.tile([1, P], F32, tag="lgc")
        nc.scalar.activation(
            out=lgc[:], in_=pslgc[0:1, 0:P], func=ACT.Identity,
            bias=b2_sb[:, 0:1], scale=1.0,
        )
        # transpose exact logits [1, CAP] -> [CAP, 1]
        ps_lgT = psM.tile([P, 512], F32, tag="mlp")
        nc.tensor.matmul(ps_lgT[:, 0:1], lhsT=lgc[:], rhs=ones_1x1[:], start=True, stop=True)
        lgcol = small.tile([CAP, 1], F32, tag="lgcol")
        nc.vector.tensor_copy(out=lgcol[:], in_=ps_lgT[:, 0:1])
        # distribute back to token positions: v[p,c] = sum_s sel[p,c,s]*lgcol[s]
        ps_v = psM.tile([P, 512], F32, tag="mlp")
        for c in range(CPT):
            ps_st = psT.tile([P, 512], F32R, tag="trR")
            nc.tensor.transpose(out=ps_st[:, 0:P].bitcast(F32),
                                in_=sel[:, c, :], identity=ident[:])
            selT = small.tile([P, P], F32, tag="selT")
            nc.vector.tensor_copy(out=selT[:], in_=ps_st[:, 0:P].bitcast(F32))
            nc.tensor.matmul(ps_v[:, c:c + 1], lhsT=selT[:], rhs=lgcol[:],
                             start=True, stop=True)
        vcorr = small.tile([P, CPT], F32, tag="vcorr")
        nc.vector.tensor_copy(out=vcorr[:], in_=ps_v[:, 0:CPT])
        # zfx = zap for non-candidates; (vcorr + logi) for candidates
        t1 = small.tile([P, CPT], F32, tag="fx_t1")
        nc.vector.tensor_tensor(out=t1[:], in0=vcorr[:], in1=logi[:], op=OP.add)
        nc.vector.tensor_tensor(out=t1[:], in0=t1[:], in1=zap[:], op=OP.subtract)
        nc.vector.tensor_tensor(out=t1[:], in0=t1[:], in1=mca[:], op=OP.mult)
        zfx = small.tile([P, CPT], F32, tag="zfx")
        nc.vector.tensor_tensor(out=zfx[:], in0=zap[:], in1=t1[:], op=OP.add)
        hard = small.tile([P, CPT], F32, tag="hard")
        nc.vector.tensor_scalar(out=hard[:], in0=zfx[:], scalar1=0.0, scalar2=None, op0=OP.is_gt)
        if dbg:
            nc.sync.dma_start(out=dbg_idx, in_=idxf[:])
            nc.sync.dma_start(out=dbg_lgcol, in_=lgcol[:])
            nc.sync.dma_start(out=dbg_lg16b, in_=vcorr[:])
            nc.sync.dma_start(out=dbg_zap, in_=zap[:])
            nc.sync.dma_start(out=dbg_hard, in_=hard[:])

        # ---- prefix-max scans: lb_inc (last boundary <= l), scan2 (boundary before it)
        def cross_part_max_scan(inclusive, tagp):
            """Combine per-partition inclusive max-scans into a global scan.

            Returns a (P, CPT) tile where each row has been max-ed with the
            running max of all previous partitions' row-maxima.
            """
            # row maxima -> (1, P) via matmul with identity rhs
            ps_r = psM.tile([P, 512], F32, tag="mlp")
            nc.tensor.matmul(
                ps_r[0:1, 0:P], lhsT=inclusive[:, CPT - 1:CPT], rhs=ident[:],
                start=True, stop=True,
            )
            rowT = small.tile([1, P], F32, tag=tagp + "_rowT")
            nc.vector.tensor_copy(out=rowT[:], in_=ps_r[0:1, 0:P])
            # inclusive scan along the (1, P) row, then shift right one (exclusive)
            sc = small.tile([1, P], F32, tag=tagp + "_sc")
            nc.vector.tensor_tensor_scan(
                out=sc[:], data0=rowT[:], data1=zrow128[:],
                initial=-1.0, op0=OP.max, op1=OP.add,
            )
            exc = small.tile([1, P], F32, tag=tagp + "_exc")
            nc.vector.memset(exc[0:1, 0:1], -1.0)
            nc.vector.tensor_copy(out=exc[0:1, 1:P], in_=sc[0:1, 0:P - 1])
            # back to (P, 1) via rank-1 matmul with ones (1,1)
            ps_b = psM.tile([P, 512], F32, tag="mlp")
            nc.tensor.matmul(
                ps_b[:, 0:1], lhsT=exc[:], rhs=ones_1x1[:], start=True, stop=True,
            )
            offm = small.tile([P, 1], F32, tag=tagp + "_offm")
            nc.vector.tensor_copy(out=offm[:], in_=ps_b[:, 0:1])
            out_t = small.tile([P, CPT], F32, tag=tagp + "_out")
            nc.vector.tensor_scalar(
                out=out_t[:], in0=inclusive[:], scalar1=offm[:], scalar2=None, op0=OP.max,
            )
            return out_t, offm

        # mi = hard ? l : -1  == (l+1)*hard - 1
        mi = small.tile([P, CPT], F32, tag="mi")
        nc.vector.tensor_tensor(out=mi[:], in0=iotp1[:], in1=hard[:], op=OP.mult)
        nc.vector.tensor_scalar(out=mi[:], in0=mi[:], scalar1=-1.0, scalar2=None, op0=OP.add)
        s1l = small.tile([P, CPT], F32, tag="s1l")
        nc.vector.tensor_tensor_scan(
            out=s1l[:], data0=mi[:], data1=zeros_cpt[:],
            initial=-1.0, op0=OP.max, op1=OP.add,
        )
        lb_inc, offm1 = cross_part_max_scan(s1l, "s1")

        # lbm1[l] = lb_inc[l-1] (token shift; layout l = p*CPT + c).
        # Column 0 of partition p is lb_inc at the end of partition p-1,
        # which is exactly the exclusive cross-partition max offm1.
        lbm1 = small.tile([P, CPT], F32, tag="lbm1")
        nc.vector.tensor_copy(out=lbm1[:, 0:1], in_=offm1[:])
        nc.vector.tensor_copy(out=lbm1[:, 1:CPT], in_=lb_inc[:, 0:CPT - 1])
        # mi2 = hard ? lbm1 : -1 == (lbm1+1)*hard - 1
        mi2 = small.tile([P, CPT], F32, tag="mi2")
        nc.vector.tensor_scalar(out=mi2[:], in0=lbm1[:], scalar1=1.0, scalar2=None, op0=OP.add)
        nc.vector.tensor_tensor(out=mi2[:], in0=mi2[:], in1=hard[:], op=OP.mult)
        nc.vector.tensor_scalar(out=mi2[:], in0=mi2[:], scalar1=-1.0, scalar2=None, op0=OP.add)
        s2l = small.tile([P, CPT], F32, tag="s2l")
        nc.vector.tensor_tensor_scan(
            out=s2l[:], data0=mi2[:], data1=zeros_cpt[:],
            initial=-1.0, op0=OP.max, op1=OP.add,
        )
        pb, _ = cross_part_max_scan(s2l, "s2")

        # cnt = lb_inc - pb ;  r = 1/(cnt + 1e-9), forced to 1.0 for null tokens
        cnt = small.tile([P, CPT], F32, tag="cnt")
        nc.vector.tensor_tensor(out=cnt[:], in0=lb_inc[:], in1=pb[:], op=OP.subtract)
        nc.vector.tensor_scalar(out=cnt[:], in0=cnt[:], scalar1=1e-9, scalar2=None, op0=OP.add)
        r_tok = small.tile([P, CPT], F32, tag="r_tok")
        nc.vector.reciprocal(out=r_tok[:], in_=cnt[:])
        mask0 = small.tile([P, CPT], F32, tag="mask0")
        nc.vector.tensor_scalar(out=mask0[:], in0=lb_inc[:], scalar1=-0.5, scalar2=None, op0=OP.is_gt)
        # r_tok = (r_tok - 1)*mask0 + 1
        nc.vector.tensor_scalar(out=r_tok[:], in0=r_tok[:], scalar1=-1.0, scalar2=None, op0=OP.add)
        nc.vector.tensor_tensor(out=r_tok[:], in0=r_tok[:], in1=mask0[:], op=OP.mult)
        nc.vector.tensor_scalar(out=r_tok[:], in0=r_tok[:], scalar1=1.0, scalar2=None, op0=OP.add)
        # pe = mask0 ? lb_inc : L+1 (null slot)   == (lb_inc - (L+1))*mask0 + (L+1)
        pe_t = small.tile([P, CPT], F32, tag="pe_t")
        nc.vector.tensor_scalar(out=pe_t[:], in0=lb_inc[:], scalar1=-float(L + 1), scalar2=None, op0=OP.add)
        nc.vector.tensor_tensor(out=pe_t[:], in0=pe_t[:], in1=mask0[:], op=OP.mult)
        nc.vector.tensor_scalar(out=pe_t[:], in0=pe_t[:], scalar1=float(L + 1), scalar2=None, op0=OP.add)
        # ps = pb >= 0 ? pb : L (zero contribution)  == (pb - L)*mask2 + L
        mask2 = small.tile([P, CPT], F32, tag="mask2")
        nc.vector.tensor_scalar(out=mask2[:], in0=pb[:], scalar1=-0.5, scalar2=None, op0=OP.is_gt)
        ps_t2 = small.tile([P, CPT], F32, tag="ps_t2")
        nc.vector.tensor_scalar(out=ps_t2[:], in0=pb[:], scalar1=-float(L), scalar2=None, op0=OP.add)
        nc.vector.tensor_tensor(out=ps_t2[:], in0=ps_t2[:], in1=mask2[:], op=OP.mult)
        nc.vector.tensor_scalar(out=ps_t2[:], in0=ps_t2[:], scalar1=float(L), scalar2=None, op0=OP.add)

        # layout bounce (p*CPT+c) -> (128t+p) chunked, all f32
        nc.sync.dma_start(out=pe_scr[:, 0].rearrange("(p c) -> p c", c=CPT), in_=pe_t[:])
        nc.sync.dma_start(out=ps_scr[:, 0].rearrange("(p c) -> p c", c=CPT), in_=ps_t2[:])
        nc.sync.dma_start(out=r_scr[:, 0].rearrange("(p c) -> p c", c=CPT), in_=r_tok[:])
        pe2 = small.tile([P, NLT], F32, tag="pe2")
        nc.sync.dma_start(out=pe2[:], in_=pe_scr[:, 0].rearrange("(t p) -> p t", p=P))
        ps2 = small.tile([P, NLT], F32, tag="ps2")
        nc.sync.dma_start(out=ps2[:], in_=ps_scr[:, 0].rearrange("(t p) -> p t", p=P))
        r2 = small.tile([P, NLT], F32, tag="r2")
        nc.sync.dma_start(out=r2[:], in_=r_scr[:, 0].rearrange("(t p) -> p t", p=P))

        # ---- final: per 128-token tile, one-hot selection matmul over the
        # window [prev tile rows | cur tile rows | null slot] of P_loc ----
        for t in range(NLT):
            base = (t - 1) * P  # global row index of window col 0

            # local window indices; clamp pe to the null slot (256).
            # pe in {0..L-1} U {L+1}; normal pe-base <= 255, null >= 257 -> 256.
            pel = gpool.tile([P, 1], F32, tag="pel")
            nc.vector.tensor_scalar(
                out=pel[:], in0=pe2[:, t:t + 1], scalar1=-float(base),
                scalar2=float(WIN - 1), op0=OP.add, op1=OP.min,
            )
            # ps in {0..L-1} U {L}; ps=L (zero row) must match nothing: no clamp
            psl = gpool.tile([P, 1], F32, tag="psl")
            nc.vector.tensor_scalar(
                out=psl[:], in0=ps2[:, t:t + 1], scalar1=-float(base),
                scalar2=None, op0=OP.add,
            )

            ct = gpool.tile([P, WIN], F32R, tag="ct")
            ct2 = gpool.tile([P, WIN], F32R, tag="ct2")
            nc.vector.tensor_scalar(out=ct[:], in0=iwin[:], scalar1=pel[:], scalar2=None, op0=OP.is_equal)
            nc.vector.tensor_scalar(out=ct2[:], in0=iwin[:], scalar1=psl[:], scalar2=None, op0=OP.is_equal)
            nc.vector.tensor_tensor(out=ct[:], in0=ct[:], in1=ct2[:], op=OP.subtract)

            if t > 0:
                # cross-tile carry: pe in cur tile & ps in prev tile ->
                # +1 on the last row of the prev tile (adds its row-sum,
                # i.e. the inter-tile offset difference)
                crA = gpool.tile([P, 1], F32R, tag="crA")
                nc.vector.tensor_scalar(out=crA[:], in0=pe2[:, t:t + 1],
                                        scalar1=float(base + P), scalar2=None, op0=OP.is_ge)
                crB = gpool.tile([P, 1], F32R, tag="crB")
                nc.vector.tensor_scalar(out=crB[:], in0=ps2[:, t:t + 1],
                                        scalar1=float(base + P), scalar2=None, op0=OP.is_lt)
                nc.vector.tensor_tensor(out=crA[:], in0=crA[:], in1=crB[:], op=OP.mult)
                nc.vector.tensor_tensor(out=ct[:, P - 1:P], in0=ct[:, P - 1:P], in1=crA[:], op=OP.add)

            # transpose CT -> C chunks (lhsT for the selection matmuls)
            ps_c = psT.tile([P, 512], F32R, tag="trR")
            if t > 0:
                nc.tensor.transpose(out=ps_c[:, 0:P], in_=ct[:, 0:P], identity=ident_r[:])
            nc.tensor.transpose(out=ps_c[:, P:2 * P], in_=ct[:, P:2 * P], identity=ident_r[:])
            if t == 0:
                nc.tensor.transpose(out=ps_c[0:1, 2 * P:3 * P], in_=ct[:, 2 * P:2 * P + 1],
                                    identity=ident_r[:])
            c_sb = gpool.tile([P, 3 * P], F32R, tag="c_sb")
            if t > 0:
                nc.vector.tensor_copy(out=c_sb[:, 0:2 * P], in_=ps_c[:, 0:2 * P])
            else:
                nc.vector.tensor_copy(out=c_sb[:, P:2 * P], in_=ps_c[:, P:2 * P])
                nc.vector.tensor_copy(out=c_sb[0:1, 2 * P:3 * P], in_=ps_c[0:1, 2 * P:3 * P])

            psO = psA.tile([P, D], F32, tag="P")
            for dc in range(NDC):
                sl = slice(dc * DC, (dc + 1) * DC)
                if t > 0:
                    nc.tensor.matmul(psO[:, sl], lhsT=c_sb[:, 0:P],
                                     rhs=pbig[:, t - 1, sl],
                                     start=True, stop=False)
                    nc.tensor.matmul(psO[:, sl], lhsT=c_sb[:, P:2 * P],
                                     rhs=pbig[:, t, sl],
                                     start=False, stop=True)
                else:
                    nc.tensor.matmul(psO[:, sl], lhsT=c_sb[:, P:2 * P],
                                     rhs=pbig[:, t, sl],
                                     start=True, stop=False)
                    nc.tensor.matmul(psO[:, sl], lhsT=c_sb[0:1, 2 * P:3 * P],
                                     rhs=null_r[0:1, sl],
                                     start=False, stop=True)

            upt = stage.tile([P, D], F32, tag="up")
            nc.scalar.activation(out=upt[:], in_=psO[:], func=ACT.Copy,
                                 bias=0.0, scale=r2[:, t:t + 1])
            nc.sync.dma_start(out=up_d[t * P:(t + 1) * P, :], in_=upt[:])

    nc.compile()
    return nc


_nc_cache = {}


def _get_nc(L, D):
    key = (L, D)
    if key not in _nc_cache:
        _nc_cache[key] = build(L, D)
    return _nc_cache[key]


def make_in_maps(inputs, n_cores=N_CORES):
    bf16 = ml_dtypes.bfloat16
    x = np.ascontiguousarray(np.asarray(inputs["x"], dtype=np.float32))
    noise = np.ascontiguousarray(np.asarray(inputs["noise"], dtype=np.float32))
    w1 = np.ascontiguousarray(np.asarray(inputs["W1"], dtype=np.float32))
    w1h = w1.astype(bf16)
    w1l = (w1 - w1h.astype(np.float32)).astype(bf16)
    shared = {
        "w1h": np.ascontiguousarray(w1h),
        "w1l": np.ascontiguousarray(w1l),
        "b1": np.ascontiguousarray(np.asarray(inputs["b1"], dtype=np.float32)),
        "W2": np.ascontiguousarray(np.asarray(inputs["W2"], dtype=np.float32)),
        "b2": np.ascontiguousarray(np.asarray(inputs["b2"], dtype=np.float32)),
        "null_group": np.ascontiguousarray(np.asarray(inputs["null_group"], dtype=np.float32)),
    }
    maps = []
    for c in range(n_cores):
        xth = np.ascontiguousarray(x[c].T.astype(bf16))   # [D, L] bf16
        maps.append(dict(shared, x=x[c], noise=noise[c], xth=xth))
    return maps


def kernel(**inputs):
    from concourse.bass_utils import run_bass_kernel_spmd

    x = np.asarray(inputs["x"])
    b, L, D = x.shape
    assert b == N_CORES
    nc = _get_nc(L, D)
    in_maps = make_in_maps(inputs)
    res = run_bass_kernel_spmd(nc, in_maps, core_ids=list(range(N_CORES)))
    out = np.stack([res.results[c]["up"] for c in range(N_CORES)], axis=0)
    return out.astype(np.float32)


# revision 27
# speedup vs baseline: 2.1369x; 1.1803x over previous
"""Trainium2 Bass kernel for the Nawrot downsampler-upsampler module.

Per-core (data-parallel over batch, 1 example per NeuronCore):
  1. PE per-tile prefix sums of x along L (triangular fp32r matmul, NO
     cross-tile carry: the carry is folded into the final selection matmul
     as a +1 coefficient on the last row of the previous tile) -> P_loc
     kept in SBUF.
  2. MLP relu(x@W1+b1) via 3-pass split-bf16 matmuls (x = xh+xl, W1 =
     wh+wl; xh*wh + xh*wl + xl*wh reproduces fp32 to ~2^-16) over
     host-pretransposed xT; logits via ones-matmul partition reduce.
  3. boundary bits from logits + logistic noise; segment scans via
     tensor_tensor_scan + cross-partition max combine.
  4. final: per 128-token tile, build a +-1 one-hot coefficient matrix
     (segment end minus segment start-1, window = prev tile + cur tile +
     null slot) and contract it against the SBUF-resident P_loc tiles with
     fp32r matmuls; scale by 1/count on the scalar engine.

fp32r (4-byte, ~12 mantissa bits, 1 cycle/row vs fp32's 4) is used where
the ~2e-4 rounding is harmless (segment averages); the logits path that
decides boundary bits needs ~1e-5 accuracy (min |logit+logistic| = 8e-5)
and uses the split-bf16 scheme instead.
"""
import sys

sys.path.insert(0, "/opt/trn_rl_repo")

import numpy as np
import ml_dtypes
from contextlib import ExitStack

import concourse.bass as bass
import concourse.bacc as bacc
import concourse.tile as tile
from concourse import mybir
from concourse.masks import make_identity

F32 = mybir.dt.float32
F32R = mybir.dt.float32r
BF16 = mybir.dt.bfloat16
OP = mybir.AluOpType
ACT = mybir.ActivationFunctionType

B = 8
L_FULL = 2048
D_FULL = 1024
N_CORES = 8


def build(L=L_FULL, D=D_FULL, dbg=False):
    P = 128
    NLT = L // P          # 128-row l-tiles
    ND = D // P           # d-tiles
    CPT = L // P          # scan columns per partition (l = p*CPT + c)
    DC = min(512, D)      # matmul free-dim chunk
    NDC = D // DC
    LCH = min(512, L)     # l-chunk for MLP matmuls
    NLC = L // LCH
    WIN = 257             # selection window: prev tile (128) + cur (128) + null
    CAP = 128             # fixup slots for near-threshold logit recompute
    TAU = 0.02            # |logit+logistic| band needing exact recompute

    nc = bacc.Bacc("TRN2", target_bir_lowering=False, debug=False, num_devices=N_CORES)

    x_d = nc.dram_tensor("x", [L, D], F32, kind="ExternalInput").ap()
    xth_d = nc.dram_tensor("xth", [D, L], BF16, kind="ExternalInput").ap()
    noise_d = nc.dram_tensor("noise", [L], F32, kind="ExternalInput").ap()
    w1h_d = nc.dram_tensor("w1h", [D, D], BF16, kind="ExternalInput").ap()
    w1l_d = nc.dram_tensor("w1l", [D, D], BF16, kind="ExternalInput").ap()
    b1_d = nc.dram_tensor("b1", [D], F32, kind="ExternalInput").ap()
    w2_d = nc.dram_tensor("W2", [D], F32, kind="ExternalInput").ap()
    b2_d = nc.dram_tensor("b2", [1], F32, kind="ExternalInput").ap()
    null_d = nc.dram_tensor("null_group", [1, 1, D], F32, kind="ExternalInput").ap()
    up_d = nc.dram_tensor("up", [L, D], F32, kind="ExternalOutput").ap()
    if dbg:
        dbg_idx = nc.dram_tensor("dbg_idx", [128, 1], F32, kind="ExternalOutput").ap()
        dbg_lgcol = nc.dram_tensor("dbg_lgcol", [128, 1], F32, kind="ExternalOutput").ap()
        dbg_lg16b = nc.dram_tensor("dbg_lg16b", [128, 16], F32, kind="ExternalOutput").ap()
        dbg_zap = nc.dram_tensor("dbg_zap", [128, 16], F32, kind="ExternalOutput").ap()
        dbg_hard = nc.dram_tensor("dbg_hard", [128, 16], F32, kind="ExternalOutput").ap()
        dbg_oi = nc.dram_tensor("dbg_oi", [128, 16], mybir.dt.int32, kind="ExternalOutput").ap()

    with tile.TileContext(nc) as tc, ExitStack() as ctx:
        const = ctx.enter_context(tc.tile_pool(name="const", bufs=1))
        dram = ctx.enter_context(tc.tile_pool(name="dram", bufs=1, space="DRAM"))
        xpool = ctx.enter_context(tc.tile_pool(name="xp", bufs=3))
        xrp = ctx.enter_context(tc.tile_pool(name="xr", bufs=2))
        xtp = ctx.enter_context(tc.tile_pool(name="xtp", bufs=2))
        stage = ctx.enter_context(tc.tile_pool(name="stage", bufs=3))
        small = ctx.enter_context(tc.tile_pool(name="small", bufs=1))
        gpool = ctx.enter_context(tc.tile_pool(name="gp", bufs=2))
        logp = ctx.enter_context(tc.tile_pool(name="logp", bufs=2))
        psA = ctx.enter_context(tc.tile_pool(name="psA", bufs=2, space="PSUM"))
        psT = ctx.enter_context(tc.tile_pool(name="psT", bufs=2, space="PSUM"))
        psM = ctx.enter_context(tc.tile_pool(name="psM", bufs=2, space="PSUM"))

        # ---------------- DRAM scratch ----------------
        lg_scr = dram.tile([L, 1], F32)      # logits row bounce
        pe_scr = dram.tile([L, 1], F32)      # per-token segment-end row index
        ps_scr = dram.tile([L, 1], F32)      # per-token segment-start-minus-one row index
        r_scr = dram.tile([L, 1], F32)       # per-token reciprocal count

        # ---------------- constants ----------------
        # (w1h/w1l DMAs are issued after the all-engine barrier so the x /
        # prefix pipeline is not gated on the 4 MB weight load)
        const_dmas = []
        w1h_sb = const.tile([P, ND, D], BF16)
        w1l_sb = const.tile([P, ND, D], BF16)
        b1_sb = const.tile([P, ND], F32)
        const_dmas.append(nc.sync.dma_start(out=b1_sb[:], in_=b1_d.rearrange("(o p) -> p o", p=P)))
        w2_sb = const.tile([P, ND], F32)
        const_dmas.append(nc.sync.dma_start(out=w2_sb[:], in_=w2_d.rearrange("(o p) -> p o", p=P)))
        b2_sb = const.tile([1, 1], F32)
        const_dmas.append(nc.sync.dma_start(out=b2_sb[:], in_=b2_d.rearrange("(a b) -> a b", a=1)))
        null_sb = const.tile([1, D], F32)
        const_dmas.append(nc.sync.dma_start(out=null_sb[:], in_=null_d[0, 0, :].rearrange("(a d) -> a d", a=1)))

        # P_loc prefix tiles, SBUF-resident across the whole kernel.
        # fp32r: consumed (pre-rounded) by the final selection matmuls.
        pbig = const.tile([P, NLT, D], F32R)

        ident = const.tile([P, P], F32)
        make_identity(nc, ident[:])
        ident_r = const.tile([P, P], F32R)
        nc.vector.tensor_copy(out=ident_r[:], in_=ident[:])

        pio = const.tile([P, 1], F32)
        nc.gpsimd.iota(pio[:], pattern=[[0, 1]], base=0, channel_multiplier=1,
                       allow_small_or_imprecise_dtypes=True)
        fio = const.tile([P, P], F32)
        nc.gpsimd.iota(fio[:], pattern=[[1, P]], base=0, channel_multiplier=0,
                       allow_small_or_imprecise_dtypes=True)
        # ut[k, m] = 1 if k <= m   (inclusive prefix lhsT), fp32r for the
        # prefix matmuls
        ut = const.tile([P, P], F32R)
        nc.vector.tensor_scalar(out=ut[:], in0=fio[:], scalar1=pio[:], scalar2=None, op0=OP.is_ge)
        ones_col = const.tile([P, 1], F32)
        nc.vector.memset(ones_col[:], 1.0)
        ones_1x1 = const.tile([1, 1], F32)
        nc.vector.memset(ones_1x1[:], 1.0)
        zeros_cpt = const.tile([P, CPT], F32)
        nc.vector.memset(zeros_cpt[:], 0.0)
        zrow128 = const.tile([1, P], F32)
        nc.vector.memset(zrow128[:], 0.0)
        iotp1 = const.tile([P, CPT], F32)   # l + 1 (l = p*CPT + c), exact in f32
        nc.gpsimd.iota(iotp1[:], pattern=[[1, CPT]], base=1, channel_multiplier=CPT,
                       allow_small_or_imprecise_dtypes=True)
        # window iota: cols 0..255 hold 0..255 (local row index), col 256 is the
        # null slot (matched by clamped pe for null tokens)
        iwin = const.tile([P, WIN], F32)
        nc.gpsimd.iota(iwin[:], pattern=[[1, WIN]], base=0, channel_multiplier=0,
                       allow_small_or_imprecise_dtypes=True)
        # null row rounded to fp32r for the t=0 selection matmul
        null_r = const.tile([1, D], F32R)
        nc.vector.tensor_copy(out=null_r[:], in_=null_sb[:])

        # Collapse the fan of constant-load DMA lanes into one tick so later
        # matmuls don't exceed the per-instruction sync-wait slot limit.  The
        # barrier NOP itself is subject to the same limit, so first absorb the
        # DMA-lane semaphores into the SP clock with nops of <=4 deps each.
        from concourse.tile_rust import add_dep_helper as _adh
        for g in range(0, len(const_dmas), 4):
            spn = nc.sync.nop()
            for d in const_dmas[g:g + 4]:
                _adh(spn.ins, d.ins, sync=True, reason="const-lane coalesce")

        # chunk-0 critical-path data goes on the DMA queues ahead of the 2 MB
        # weight load; w1l (first needed in the fixup, ~150us in) is issued
        # after chunk 0 to keep startup bandwidth for the x / xth pipeline.
        xth_first = xtp.tile([P, ND, LCH], BF16, tag="xTh")
        nc.sync.dma_start(out=xth_first[:], in_=xth_d[:, 0:LCH].rearrange("(i p) l -> p i l", p=P))
        x_first = xpool.tile([P, D], F32, tag="x")
        nc.sync.dma_start(out=x_first[:], in_=x_d[0:P, :])

        tc.strict_bb_all_engine_barrier()

        nc.sync.dma_start(out=w1h_sb[:], in_=w1h_d.rearrange("(i p) n -> p i n", p=P))

        # shared bounds-check registers for the fixup's indirect DMAs
        bc_cap = nc.gpsimd.to_reg(CAP - 1)
        bc_row = nc.gpsimd.to_reg(L - 1)

        # ------- phases 1+2: per 512-token chunk: load, prefix, MLP -------
        for lc in range(NLC):
            lsl = slice(lc * LCH, (lc + 1) * LCH)
            if lc == 0:
                xth_ch = xth_first
            else:
                xth_ch = xtp.tile([P, ND, LCH], BF16, tag="xTh")
                nc.sync.dma_start(out=xth_ch[:], in_=xth_d[:, lsl].rearrange("(i p) l -> p i l", p=P))
            if lc == 1:
                nc.sync.dma_start(out=w1l_sb[:], in_=w1l_d.rearrange("(i p) n -> p i n", p=P))

            for ii in range(LCH // P):
                i = lc * (LCH // P) + ii
                if i == 0:
                    x_t = x_first
                else:
                    x_t = xpool.tile([P, D], F32, tag="x")
                    nc.sync.dma_start(out=x_t[:], in_=x_d[i * P:(i + 1) * P, :])
                # round to fp32r for the prefix matmul
                x_r = xrp.tile([P, D], F32R, tag="xr")
                nc.vector.tensor_copy(out=x_r[:], in_=x_t[:])

                # per-tile prefix (no cross-tile carry; folded into phase 4)
                psP = psA.tile([P, D], F32, tag="P")
                for dc in range(NDC):
                    sl = slice(dc * DC, (dc + 1) * DC)
                    nc.tensor.matmul(
                        psP[:, sl], lhsT=ut[:], rhs=x_r[:, sl],
                        start=True, stop=True,
                    )
                nc.vector.tensor_copy(out=pbig[:, i, :], in_=psP[:])

            # MLP pass A for this l-chunk: single bf16 pass (xh*wh).  Logit
            # error <= ~7e-3; tokens within TAU of the boundary threshold are
            # recomputed exactly in the fixup phase below.
            logacc = logp.tile([P, LCH], F32, tag="logacc")
            for o in range(ND):
                psm = psM.tile([P, LCH], F32, tag="mlp")
                for i_ in range(ND):
                    nc.tensor.matmul(
                        psm[:],
                        lhsT=w1h_sb[:, i_, o * P:(o + 1) * P],
                        rhs=xth_ch[:, i_, :],
                        start=(i_ == 0), stop=(i_ == ND - 1),
                    )
                hT = stage.tile([P, LCH], F32, tag="hT")
                nc.scalar.activation(
                    out=hT[:], in_=psm[:], func=ACT.Relu,
                    bias=b1_sb[:, o:o + 1], scale=1.0,
                )
                if o == 0:
                    nc.vector.tensor_scalar(
                        out=logacc[:], in0=hT[:],
                        scalar1=w2_sb[:, o:o + 1], scalar2=None, op0=OP.mult,
                    )
                else:
                    nc.vector.scalar_tensor_tensor(
                        out=logacc[:], in0=hT[:], scalar=w2_sb[:, o:o + 1],
                        in1=logacc[:], op0=OP.mult, op1=OP.add,
                    )

            # logits partial for this chunk: partition-reduce + bias, to DRAM
            pslg = psM.tile([1, LCH], F32, tag="mlp")
            nc.tensor.matmul(pslg[:], lhsT=ones_col[:], rhs=logacc[:], start=True, stop=True)
            lg_ch = stage.tile([1, LCH], F32, tag="lgch")
            nc.scalar.activation(
                out=lg_ch[:], in_=pslg[:], func=ACT.Identity,
                bias=b2_sb[:, 0:1], scale=1.0,
            )
            nc.sync.dma_start(
                out=lg_scr[lsl, 0].rearrange("(a l) -> a l", a=1), in_=lg_ch[:]
            )

        # ---------------- phase 3: boundary bits, cumsum ----------------
        lg16 = small.tile([P, CPT], F32, tag="lg16")
        nc.sync.dma_start(out=lg16[:], in_=lg_scr[:, 0].rearrange("(p c) -> p c", c=CPT))

        nz16 = small.tile([P, CPT], F32, tag="nz")
        nc.sync.dma_start(out=nz16[:], in_=noise_d.rearrange("(p c) -> p c", c=CPT))

        lnu = small.tile([P, CPT], F32, tag="lnu")
        nc.scalar.activation(out=lnu[:], in_=nz16[:], func=ACT.Ln)
        om = small.tile([P, CPT], F32, tag="om")
        nc.vector.tensor_scalar(
            out=om[:], in0=nz16[:], scalar1=1.0, scalar2=-1.0,
            op0=OP.subtract, op1=OP.mult,
        )  # (u - 1) * -1 = 1 - u
        ln1m = small.tile([P, CPT], F32, tag="ln1m")
        nc.scalar.activation(out=ln1m[:], in_=om[:], func=ACT.Ln)
        logi = small.tile([P, CPT], F32, tag="logi")
        nc.vector.tensor_tensor(out=logi[:], in0=lnu[:], in1=ln1m[:], op=OP.subtract)
        zap = small.tile([P, CPT], F32, tag="zap")
        nc.vector.tensor_tensor(out=zap[:], in0=logi[:], in1=lg16[:], op=OP.add)

        # ---- logit fixup: exactly recompute tokens with |z| < TAU ----
        # candidate mask
        msq = small.tile([P, CPT], F32, tag="msq")
        nc.vector.tensor_tensor(out=msq[:], in0=zap[:], in1=zap[:], op=OP.mult)
        mca = small.tile([P, CPT], F32, tag="mca")
        nc.vector.tensor_scalar(out=mca[:], in0=msq[:], scalar1=TAU * TAU, scalar2=None, op0=OP.is_lt)
        # global exclusive rank of each candidate (token order l = p*CPT + c)
        minc = small.tile([P, CPT], F32, tag="minc")
        nc.vector.tensor_tensor_scan(
            out=minc[:], data0=mca[:], data1=zeros_cpt[:],
            initial=0.0, op0=OP.add, op1=OP.add,
        )
        ps_r2 = psM.tile([P, 512], F32, tag="mlp")
        nc.tensor.matmul(ps_r2[0:1, 0:P], lhsT=minc[:, CPT - 1:CPT], rhs=ident[:],
                         start=True, stop=True)
        rowT2 = small.tile([1, P], F32, tag="fx_rowT")
        nc.vector.tensor_copy(out=rowT2[:], in_=ps_r2[0:1, 0:P])
        sc2t = small.tile([1, P], F32, tag="fx_sc")
        nc.vector.tensor_tensor_scan(
            out=sc2t[:], data0=rowT2[:], data1=zrow128[:],
            initial=0.0, op0=OP.add, op1=OP.add,
        )
        exc2 = small.tile([1, P], F32, tag="fx_exc")
        nc.vector.memset(exc2[0:1, 0:1], 0.0)
        nc.vector.tensor_copy(out=exc2[0:1, 1:P], in_=sc2t[0:1, 0:P - 1])
        ps_b2 = psM.tile([P, 512], F32, tag="mlp")
        nc.tensor.matmul(ps_b2[:, 0:1], lhsT=exc2[:], rhs=ones_1x1[:], start=True, stop=True)
        offc = small.tile([P, 1], F32, tag="fx_offc")
        nc.vector.tensor_copy(out=offc[:], in_=ps_b2[:, 0:1])
        rank = small.tile([P, CPT], F32, tag="rank")
        nc.vector.tensor_scalar(out=rank[:], in0=minc[:], scalar1=offc[:], scalar2=None, op0=OP.add)
        nc.vector.tensor_tensor(out=rank[:], in0=rank[:], in1=mca[:], op=OP.subtract)
        candl = small.tile([P, CPT], F32, tag="candl")
        nc.vector.tensor_scalar(out=candl[:], in0=iotp1[:], scalar1=-1.0, scalar2=None, op0=OP.add)

        # slot-selection one-hots: sel[p, c, s] = (rank[p,c]==s) * mca[p,c].
        # Slot index of every candidate in [0, CAP); all-zero columns for
        # unused slots (those slots resolve to token 0, recomputed harmlessly).
        sel = small.tile([P, CPT, P], F32, tag="sel")
        for c in range(CPT):
            nc.vector.tensor_scalar(
                out=sel[:, c, :], in0=fio[:], scalar1=rank[:, c:c + 1],
                scalar2=mca[:, c:c + 1], op0=OP.is_equal, op1=OP.mult,
            )
        # idx[s] = sum_{p,c} sel[p,c,s] * token_index[p,c]  (PE contraction)
        ps_idx = psM.tile([P, 512], F32, tag="mlp")
        for c in range(CPT):
            nc.tensor.matmul(
                ps_idx[:, 0:1], lhsT=sel[:, c, :], rhs=candl[:, c:c + 1],
                start=(c == 0), stop=(c == CPT - 1),
            )
        idxf = small.tile([CAP, 1], F32, tag="idxf")
        nc.vector.tensor_copy(out=idxf[:], in_=ps_idx[:, 0:1])
        idxi = small.tile([CAP, 1], mybir.dt.int32, tag="idxi")
        nc.vector.tensor_copy(out=idxi[:], in_=idxf[:])

        # gather candidate token rows of x, transpose, split hi/lo bf16
        xcand = small.tile([P, D], F32, tag="xcand")
        nc.gpsimd.indirect_dma_start(
            out=xcand[:], out_offset=None, in_=x_d,
            in_offset=bass.IndirectOffsetOnAxis(ap=idxi[:], axis=0),
            bounds_check=bc_row, oob_is_err=False,
        )
        xtc_h = small.tile([P, ND, P], BF16, tag="xtc_h")
        xtc_l = small.tile([P, ND, P], BF16, tag="xtc_l")
        for g in range(2):
            psc_t = psM.tile([P, 512], F32, tag="mlp")
            for jj in range(4):
                j = g * 4 + jj
                nc.tensor.transpose(
                    out=psc_t[:, jj * P:(jj + 1) * P],
                    in_=xcand[:, j * P:(j + 1) * P],
                    identity=ident[:],
                )
            pview = psc_t[:, 0:512].rearrange("p (j q) -> p j q", q=P)
            nc.vector.tensor_copy(out=xtc_h[:, g * 4:g * 4 + 4, :], in_=pview)
            tmp32 = small.tile([P, 512], F32, tag="tmp32")
            tview = tmp32[:, 0:512].rearrange("p (j q) -> p j q", q=P)
            nc.vector.tensor_copy(out=tview, in_=xtc_h[:, g * 4:g * 4 + 4, :])
            nc.vector.tensor_tensor(out=xtc_l[:, g * 4:g * 4 + 4, :], in0=pview,
                                    in1=tview, op=OP.subtract)

        # exact 3-pass MLP on the candidate columns
        logacc_c = small.tile([P, P], F32, tag="logc")
        for o in range(ND):
            psc2 = psM.tile([P, 512], F32, tag="mlp")
            n_mm = ND * 3
            k = 0
            for i_ in range(ND):
                for lhs_t, rhs_t in (
                    (w1h_sb, xtc_h), (w1l_sb, xtc_h), (w1h_sb, xtc_l),
                ):
                    nc.tensor.matmul(
                        psc2[:, 0:P],
                        lhsT=lhs_t[:, i_, o * P:(o + 1) * P],
                        rhs=rhs_t[:, i_, :],
                        start=(k == 0), stop=(k == n_mm - 1),
                    )
                    k += 1
            hTc = small.tile([P, P], F32, tag="hTc")
            nc.scalar.activation(
                out=hTc[:], in_=psc2[:, 0:P], func=ACT.Relu,
                bias=b1_sb[:, o:o + 1], scale=1.0,
            )
            if o == 0:
                nc.vector.tensor_scalar(
                    out=logacc_c[:], in0=hTc[:],
                    scalar1=w2_sb[:, o:o + 1], scalar2=None, op0=OP.mult,
                )
            else:
                nc.vector.scalar_tensor_tensor(
                    out=logacc_c[:], in0=hTc[:], scalar=w2_sb[:, o:o + 1],
                    in1=logacc_c[:], op0=OP.mult, op1=OP.add,
                )
        pslgc = psM.tile([P, 512], F32, tag="mlp")
        nc.tensor.matmul(pslgc[0:1, 0:P], lhsT=ones_col[:], rhs=logacc_c[:],
                         start=True, stop=True)
        lgc = small.tile([1, P], F32, tag="lgc")
        nc.scalar.activation(
            out=lgc[:], in_=pslgc[0:1, 0:P], func=ACT.Identity,
            bias=b2_sb[:, 0:1], scale=1.0,
        )
        # transpose exact logits [1, CAP] -> [CAP, 1]
        ps_lgT = psM.tile([P, 512], F32, tag="mlp")
        nc.tensor.matmul(ps_lgT[:, 0:1], lhsT=lgc[:], rhs=ones_1x1[:], start=True, stop=True)
        lgcol = small.tile([CAP, 1], F32, tag="lgcol")
        nc.vector.tensor_copy(out=lgcol[:], in_=ps_lgT[:, 0:1])
        # distribute back to token positions: v[p,c] = sum_s sel[p,c,s]*lgcol[s]
        ps_v = psM.tile([P, 512], F32, tag="mlp")
        for c in range(CPT):
            ps_st = psT.tile([P, 512], F32R, tag="trR")
            nc.tensor.transpose(out=ps_st[:, 0:P].bitcast(F32),
                                in_=sel[:, c, :], identity=ident[:])
            selT = small.tile([P, P], F32, tag="selT")
            nc.vector.tensor_copy(out=selT[:], in_=ps_st[:, 0:P].bitcast(F32))
            nc.tensor.matmul(ps_v[:, c:c + 1], lhsT=selT[:], rhs=lgcol[:],
                             start=True, stop=True)
        vcorr = small.tile([P, CPT], F32, tag="vcorr")
        nc.vector.tensor_copy(out=vcorr[:], in_=ps_v[:, 0:CPT])
        # zfx = zap for non-candidates; (vcorr + logi) for candidates
        t1 = small.tile([P, CPT], F32, tag="fx_t1")
        nc.vector.tensor_tensor(out=t1[:], in0=vcorr[:], in1=logi[:], op=OP.add)
        nc.vector.tensor_tensor(out=t1[:], in0=t1[:], in1=zap[:], op=OP.subtract)
        nc.vector.tensor_tensor(out=t1[:], in0=t1[:], in1=mca[:], op=OP.mult)
        zfx = small.tile([P, CPT], F32, tag="zfx")
        nc.vector.tensor_tensor(out=zfx[:], in0=zap[:], in1=t1[:], op=OP.add)
        hard = small.tile([P, CPT], F32, tag="hard")
        nc.vector.tensor_scalar(out=hard[:], in0=zfx[:], scalar1=0.0, scalar2=None, op0=OP.is_gt)
        if dbg:
            nc.sync.dma_start(out=dbg_idx, in_=idxf[:])
            nc.sync.dma_start(out=dbg_lgcol, in_=lgcol[:])
            nc.sync.dma_start(out=dbg_lg16b, in_=vcorr[:])
            nc.sync.dma_start(out=dbg_zap, in_=zap[:])
            nc.sync.dma_start(out=dbg_hard, in_=hard[:])

        # ---- prefix-max scans: lb_inc (last boundary <= l), scan2 (boundary before it)
        def cross_part_max_scan(inclusive, tagp):
            """Combine per-partition inclusive max-scans into a global scan.

            Returns a (P, CPT) tile where each row has been max-ed with the
            running max of all previous partitions' row-maxima.
            """
            # row maxima -> (1, P) via matmul with identity rhs
            ps_r = psM.tile([P, 512], F32, tag="mlp")
            nc.tensor.matmul(
                ps_r[0:1, 0:P], lhsT=inclusive[:, CPT - 1:CPT], rhs=ident[:],
                start=True, stop=True,
            )
            rowT = small.tile([1, P], F32, tag=tagp + "_rowT")
            nc.vector.tensor_copy(out=rowT[:], in_=ps_r[0:1, 0:P])
            # inclusive scan along the (1, P) row, then shift right one (exclusive)
            sc = small.tile([1, P], F32, tag=tagp + "_sc")
            nc.vector.tensor_tensor_scan(
                out=sc[:], data0=rowT[:], data1=zrow128[:],
                initial=-1.0, op0=OP.max, op1=OP.add,
            )
            exc = small.tile([1, P], F32, tag=tagp + "_exc")
            nc.vector.memset(exc[0:1, 0:1], -1.0)
            nc.vector.tensor_copy(out=exc[0:1, 1:P], in_=sc[0:1, 0:P - 1])
            # back to (P, 1) via rank-1 matmul with ones (1,1)
            ps_b = psM.tile([P, 512], F32, tag="mlp")
            nc.tensor.matmul(
                ps_b[:, 0:1], lhsT=exc[:], rhs=ones_1x1[:], start=True, stop=True,
            )
            offm = small.tile([P, 1], F32, tag=tagp + "_offm")
            nc.vector.tensor_copy(out=offm[:], in_=ps_b[:, 0:1])
            out_t = small.tile([P, CPT], F32, tag=tagp + "_out")
            nc.vector.tensor_scalar(
                out=out_t[:], in0=inclusive[:], scalar1=offm[:], scalar2=None, op0=OP.max,
            )
            return out_t, offm

        # mi = hard ? l : -1  == (l+1)*hard - 1
        mi = small.tile([P, CPT], F32, tag="mi")
        nc.vector.tensor_tensor(out=mi[:], in0=iotp1[:], in1=hard[:], op=OP.mult)
        nc.vector.tensor_scalar(out=mi[:], in0=mi[:], scalar1=-1.0, scalar2=None, op0=OP.add)
        s1l = small.tile([P, CPT], F32, tag="s1l")
        nc.vector.tensor_tensor_scan(
            out=s1l[:], data0=mi[:], data1=zeros_cpt[:],
            initial=-1.0, op0=OP.max, op1=OP.add,
        )
        lb_inc, offm1 = cross_part_max_scan(s1l, "s1")

        # lbm1[l] = lb_inc[l-1] (token shift; layout l = p*CPT + c).
        # Column 0 of partition p is lb_inc at the end of partition p-1,
        # which is exactly the exclusive cross-partition max offm1.
        lbm1 = small.tile([P, CPT], F32, tag="lbm1")
        nc.vector.tensor_copy(out=lbm1[:, 0:1], in_=offm1[:])
        nc.vector.tensor_copy(out=lbm1[:, 1:CPT], in_=lb_inc[:, 0:CPT - 1])
        # mi2 = hard ? lbm1 : -1 == (lbm1+1)*hard - 1
        mi2 = small.tile([P, CPT], F32, tag="mi2")
        nc.vector.tensor_scalar(out=mi2[:], in0=lbm1[:], scalar1=1.0, scalar2=None, op0=OP.add)
        nc.vector.tensor_tensor(out=mi2[:], in0=mi2[:], in1=hard[:], op=OP.mult)
        nc.vector.tensor_scalar(out=mi2[:], in0=mi2[:], scalar1=-1.0, scalar2=None, op0=OP.add)
        s2l = small.tile([P, CPT], F32, tag="s2l")
        nc.vector.tensor_tensor_scan(
            out=s2l[:], data0=mi2[:], data1=zeros_cpt[:],
            initial=-1.0, op0=OP.max, op1=OP.add,
        )
        pb, _ = cross_part_max_scan(s2l, "s2")

        # cnt = lb_inc - pb ;  r = 1/(cnt + 1e-9), forced to 1.0 for null tokens
        cnt = small.tile([P, CPT], F32, tag="cnt")
        nc.vector.tensor_tensor(out=cnt[:], in0=lb_inc[:], in1=pb[:], op=OP.subtract)
        nc.vector.tensor_scalar(out=cnt[:], in0=cnt[:], scalar1=1e-9, scalar2=None, op0=OP.add)
        r_tok = small.tile([P, CPT], F32, tag="r_tok")
        nc.vector.reciprocal(out=r_tok[:], in_=cnt[:])
        mask0 = small.tile([P, CPT], F32, tag="mask0")
        nc.vector.tensor_scalar(out=mask0[:], in0=lb_inc[:], scalar1=-0.5, scalar2=None, op0=OP.is_gt)
        # r_tok = (r_tok - 1)*mask0 + 1
        nc.vector.tensor_scalar(out=r_tok[:], in0=r_tok[:], scalar1=-1.0, scalar2=None, op0=OP.add)
        nc.vector.tensor_tensor(out=r_tok[:], in0=r_tok[:], in1=mask0[:], op=OP.mult)
        nc.vector.tensor_scalar(out=r_tok[:], in0=r_tok[:], scalar1=1.0, scalar2=None, op0=OP.add)
        # pe = mask0 ? lb_inc : L+1 (null slot)   == (lb_inc - (L+1))*mask0 + (L+1)
        pe_t = small.tile([P, CPT], F32, tag="pe_t")
        nc.vector.tensor_scalar(out=pe_t[:], in0=lb_inc[:], scalar1=-float(L + 1), scalar2=None, op0=OP.add)
        nc.vector.tensor_tensor(out=pe_t[:], in0=pe_t[:], in1=mask0[:], op=OP.mult)
        nc.vector.tensor_scalar(out=pe_t[:], in0=pe_t[:], scalar1=float(L + 1), scalar2=None, op0=OP.add)
        # ps = pb >= 0 ? pb : L (zero contribution)  == (pb - L)*mask2 + L
        mask2 = small.tile([P, CPT], F32, tag="mask2")
        nc.vector.tensor_scalar(out=mask2[:], in0=pb[:], scalar1=-0.5, scalar2=None, op0=OP.is_gt)
        ps_t2 = small.tile([P, CPT], F32, tag="ps_t2")
        nc.vector.tensor_scalar(out=ps_t2[:], in0=pb[:], scalar1=-float(L), scalar2=None, op0=OP.add)
        nc.vector.tensor_tensor(out=ps_t2[:], in0=ps_t2[:], in1=mask2[:], op=OP.mult)
        nc.vector.tensor_scalar(out=ps_t2[:], in0=ps_t2[:], scalar1=float(L), scalar2=None, op0=OP.add)

        # layout bounce (p*CPT+c) -> (128t+p) chunked, all f32
        nc.sync.dma_start(out=pe_scr[:, 0].rearrange("(p c) -> p c", c=CPT), in_=pe_t[:])
        nc.sync.dma_start(out=ps_scr[:, 0].rearrange("(p c) -> p c", c=CPT), in_=ps_t2[:])
        nc.sync.dma_start(out=r_scr[:, 0].rearrange("(p c) -> p c", c=CPT), in_=r_tok[:])
        pe2 = small.tile([P, NLT], F32, tag="pe2")
        nc.sync.dma_start(out=pe2[:], in_=pe_scr[:, 0].rearrange("(t p) -> p t", p=P))
        ps2 = small.tile([P, NLT], F32, tag="ps2")
        nc.sync.dma_start(out=ps2[:], in_=ps_scr[:, 0].rearrange("(t p) -> p t", p=P))
        r2 = small.tile([P, NLT], F32, tag="r2")
        nc.sync.dma_start(out=r2[:], in_=r_scr[:, 0].rearrange("(t p) -> p t", p=P))

        # ---- final: per 128-token tile, one-hot selection matmul over the
        # window [prev tile rows | cur tile rows | null slot] of P_loc ----
        for t in range(NLT):
            base = (t - 1) * P  # global row index of window col 0

            # local window indices; clamp pe to the null slot (256).
            # pe in {0..L-1} U {L+1}; normal pe-base <= 255, null >= 257 -> 256.
            pel = gpool.tile([P, 1], F32, tag="pel")
            nc.vector.tensor_scalar(
                out=pel[:], in0=pe2[:, t:t + 1], scalar1=-float(base),
                scalar2=float(WIN - 1), op0=OP.add, op1=OP.min,
            )
            # ps in {0..L-1} U {L}; ps=L (zero row) must match nothing: no clamp
            psl = gpool.tile([P, 1], F32, tag="psl")
            nc.vector.tensor_scalar(
                out=psl[:], in0=ps2[:, t:t + 1], scalar1=-float(base),
                scalar2=None, op0=OP.add,
            )

            ct = gpool.tile([P, WIN], F32R, tag="ct")
            ct2 = gpool.tile([P, WIN], F32R, tag="ct2")
            nc.vector.tensor_scalar(out=ct[:], in0=iwin[:], scalar1=pel[:], scalar2=None, op0=OP.is_equal)
            nc.vector.tensor_scalar(out=ct2[:], in0=iwin[:], scalar1=psl[:], scalar2=None, op0=OP.is_equal)
            nc.vector.tensor_tensor(out=ct[:], in0=ct[:], in1=ct2[:], op=OP.subtract)

            if t > 0:
                # cross-tile carry: pe in cur tile & ps in prev tile ->
                # +1 on the last row of the prev tile (adds its row-sum,
                # i.e. the inter-tile offset difference)
                crA = gpool.tile([P, 1], F32R, tag="crA")
                nc.vector.tensor_scalar(out=crA[:], in0=pe2[:, t:t + 1],
                                        scalar1=float(base + P), scalar2=None, op0=OP.is_ge)
                crB = gpool.tile([P, 1], F32R, tag="crB")
                nc.vector.tensor_scalar(out=crB[:], in0=ps2[:, t:t + 1],
                                        scalar1=float(base + P), scalar2=None, op0=OP.is_lt)
                nc.vector.tensor_tensor(out=crA[:], in0=crA[:], in1=crB[:], op=OP.mult)
                nc.vector.tensor_tensor(out=ct[:, P - 1:P], in0=ct[:, P - 1:P], in1=crA[:], op=OP.add)

            # transpose CT -> C chunks (lhsT for the selection matmuls)
            ps_c = psT.tile([P, 512], F32R, tag="trR")
            if t > 0:
                nc.tensor.transpose(out=ps_c[:, 0:P], in_=ct[:, 0:P], identity=ident_r[:])
            nc.tensor.transpose(out=ps_c[:, P:2 * P], in_=ct[:, P:2 * P], identity=ident_r[:])
            if t == 0:
                nc.tensor.transpose(out=ps_c[0:1, 2 * P:3 * P], in_=ct[:, 2 * P:2 * P + 1],
                                    identity=ident_r[:])
            c_sb = gpool.tile([P, 3 * P], F32R, tag="c_sb")
            if t > 0:
                nc.vector.tensor_copy(out=c_sb[:, 0:2 * P], in_=ps_c[:, 0:2 * P])
            else:
                nc.vector.tensor_copy(out=c_sb[:, P:2 * P], in_=ps_c[:, P:2 * P])
                nc.vector.tensor_copy(out=c_sb[0:1, 2 * P:3 * P], in_=ps_c[0:1, 2 * P:3 * P])

            psO = psA.tile([P, D], F32, tag="P")
            for dc in range(NDC):
                sl = slice(dc * DC, (dc + 1) * DC)
                if t > 0:
                    nc.tensor.matmul(psO[:, sl], lhsT=c_sb[:, 0:P],
                                     rhs=pbig[:, t - 1, sl],
                                     start=True, stop=False)
                    nc.tensor.matmul(psO[:, sl], lhsT=c_sb[:, P:2 * P],
                                     rhs=pbig[:, t, sl],
                                     start=False, stop=True)
                else:
                    nc.tensor.matmul(psO[:, sl], lhsT=c_sb[:, P:2 * P],
                                     rhs=pbig[:, t, sl],
                                     start=True, stop=False)
                    nc.tensor.matmul(psO[:, sl], lhsT=c_sb[0:1, 2 * P:3 * P],
                                     rhs=null_r[0:1, sl],
                                     start=False, stop=True)

            upt = stage.tile([P, D], F32, tag="up")
            nc.scalar.activation(out=upt[:], in_=psO[:], func=ACT.Copy,
                                 bias=0.0, scale=r2[:, t:t + 1])
            nc.sync.dma_start(out=up_d[t * P:(t + 1) * P, :], in_=upt[:])

    nc.compile()
    return nc


_nc_cache = {}


def _get_nc(L, D):
    key = (L, D)
    if key not in _nc_cache:
        _nc_cache[key] = build(L, D)
    return _nc_cache[key]


def make_in_maps(inputs, n_cores=N_CORES):
    bf16 = ml_dtypes.bfloat16
    x = np.ascontiguousarray(np.asarray(inputs["x"], dtype=np.float32))
    noise = np.ascontiguousarray(np.asarray(inputs["noise"], dtype=np.float32))
    w1 = np.ascontiguousarray(np.asarray(inputs["W1"], dtype=np.float32))
    w1h = w1.astype(bf16)
    w1l = (w1 - w1h.astype(np.float32)).astype(bf16)
    shared = {
        "w1h": np.ascontiguousarray(w1h),
        "w1l": np.ascontiguousarray(w1l),
        "b1": np.ascontiguousarray(np.asarray(inputs["b1"], dtype=np.float32)),
        "W2": np.ascontiguousarray(np.asarray(inputs["W2"], dtype=np.float32)),
        "b2": np.ascontiguousarray(np.asarray(inputs["b2"], dtype=np.float32)),
        "null_group": np.ascontiguousarray(np.asarray(inputs["null_group"], dtype=np.float32)),
    }
    maps = []
    for c in range(n_cores):
        xth = np.ascontiguousarray(x[c].T.astype(bf16))   # [D, L] bf16
        maps.append(dict(shared, x=x[c], noise=noise[c], xth=xth))
    return maps


def kernel(**inputs):
    from concourse.bass_utils import run_bass_kernel_spmd

    x = np.asarray(inputs["x"])
    b, L, D = x.shape
    assert b == N_CORES
    nc = _get_nc(L, D)
    in_maps = make_in_maps(inputs)
    res = run_bass_kernel_spmd(nc, in_maps, core_ids=list(range(N_CORES)))
    out = np.stack([res.results[c]["up"] for c in range(N_CORES)], axis=0)
    return out.astype(np.float32)
